# revision 25
# baseline (speedup 1.0000x reference)
"""Bass/Trainium2 kernel for nn_AttentionCTCLoss (RAD-TTS attention CTC loss).

Pure data-parallel over 8 NeuronCores (16 samples each). Per core:
softmax over 201 classes kept UNNORMALIZED (raw exp; per-frame denominators
exported, folded out on host). The 900-step CTC alpha recursion runs in the
probability domain on a chunked layout: partitions p = b*8 + j, chunk j holds
states [52j, 52j+52) plus a 10-col ghost halo [52j-10, 52j) refreshed every 4
steps with a single DVE stream_shuffle (partition+1 copy within quadrants).
Per step the DVE does exactly 3 ops (add, strided skip-add, STT multiply by
emissions with rescale factor folded in). Per-sample rescale every 12 steps
via PE matmuls (measure accum -> G1 -> G2 -> reciprocal), applied with an
8-step lag. Final-state extraction uses an emission "freeze": for t >= ol_b
the emission row becomes one-hot 1.0 at state 2L_b, which makes
alpha[2L_b] = hi+lo and keeps it fixed, so no per-step snapshots are needed.
Host reconstructs loss_b = -(ln alpha[2L] - sum ln D_t + sum ln M_k) / L_b.
"""
import math
import numpy as np
from contextlib import ExitStack

import concourse.bass as bass
import concourse.mybir as mybir
from concourse.bass_utils import run_bass_kernel_spmd

LAST_RESULTS = None
LAST_EXEC_S = None

F32 = mybir.dt.float32
ALU = mybir.AluOpType
ACTF = mybir.ActivationFunctionType

NCORES = 8
NB = 16            # samples per core
TQ, TK = 900, 200
NCH = 8            # state chunks per sample
W = 52             # interior states per chunk (8*52 = 416 >= 401)
GW = 10            # ghost halo columns
TW = GW + W        # tile width 62
KSH = 4            # ghost refresh (stream_shuffle) period
RESC = 12          # rescale period
LAG = 8            # measure -> apply lag
EB = math.exp(-1.0)
SHMASK = [i - 1 if i % 8 else i + 7 for i in range(32)]


def _build(Tmax, measures, G, NR):
    nc = bass.Bass()
    x_d = nc.declare_dram_parameter("x", [NB, TQ, TK], F32, isOutput=False)
    odd_d = nc.declare_dram_parameter("oddm", [128, NCH, 26], F32, isOutput=False)
    evpmfz_d = nc.declare_dram_parameter("evpmfz", [128, NCH, 26], F32, isOutput=False)
    fz_d = nc.declare_dram_parameter("fzp", [128, NCH, 26], F32, isOutput=False)
    liveb_d = nc.declare_dram_parameter("liveb", [128, G], F32, isOutput=False)
    biasb_d = nc.declare_dram_parameter("biasb", [128, G], F32, isOutput=False)
    initm_d = nc.declare_dram_parameter("initm", [128, TW], F32, isOutput=False)
    g1_d = nc.declare_dram_parameter("g1", [128, NB], F32, isOutput=False)
    g2_d = nc.declare_dram_parameter("g2", [NB, 128], F32, isOutput=False)
    alpha_d = nc.declare_dram_parameter("alpha", [128, TW], F32, isOutput=True)
    outsb_d = nc.declare_dram_parameter("outsb", [NB, max(NR, 1)], F32, isOutput=True)
    ss_d = nc.declare_dram_parameter("ssden", [128, G], F32, isOutput=True)
    emit_d = nc.dram_tensor("emitd", [G, 8, NB, NCH, TW], F32)

    stack = ExitStack()
    def sb(name, shape):
        return stack.enter_context(nc.sbuf_tensor(name, shape, F32))
    XR = [sb("xr%d" % i, [128, TK]) for i in range(4)]
    ER4 = [sb("er4_%d" % i, [128, NCH * 26]) for i in range(4)]
    EMR = [sb("emr%d" % i, [128, NCH, TW]) for i in range(3)]
    ER8 = [sb("er8_%d" % i, [128, 8, TW]) for i in range(3)]
    ODD = sb("oddsb", [128, NCH, 26])
    EVPMFZ = sb("evpmfzsb", [128, NCH, 26])
    FZ = sb("fzsb", [128, NCH, 26])
    LIVEB = sb("livebsb", [128, G])
    BIASB = sb("biasbsb", [128, G])
    INITM = sb("initmsb", [128, TW])
    G1S = sb("g1sb", [128, NB])
    G2S = sb("g2sb", [NB, 128])
    AB = [sb("ab%d" % i, [128, TW]) for i in range(2)]
    MS = [sb("ms%d" % i, [128, 1]) for i in range(2)]
    INV = sb("invsb", [128, 1])
    OUTSB = sb("outsbsb", [NB, max(NR, 1)])
    SSD = sb("ssdsb", [128, G])
    PS1 = stack.enter_context(nc.psum_tensor("ps1t", [NB, 1], F32))
    PS2 = stack.enter_context(nc.psum_tensor("ps2t", [128, 1], F32))

    r_of = lambda g: min(8, Tmax - 8 * g)
    me_at = {t: m for m, t in enumerate(measures)}
    cp_at = {t + LAG // 2: m for m, t in enumerate(measures)}
    ap_at = {t + LAG: m for m, t in enumerate(measures)}

    xdma = [stack.enter_context(nc.semaphore("xdma%d" % i)) for i in range(4)]
    with (
        nc.Block() as block,
        nc.semaphore("cdma") as cdma,
        nc.semaphore("acts") as acts,
        nc.semaphore("evc") as evc,      # scalar even-pattern copy done
        nc.semaphore("pa") as pa,        # Pool finished A (ER4 consumed)
        nc.semaphore("pc") as pc,        # Pool finished tile build
        nc.semaphore("emst") as emst,    # emit store DMA complete
        nc.semaphore("elo") as elo,      # ER8 load complete
        nc.semaphore("cons") as cons,    # DVE consumed tile
        nc.semaphore("msem") as msem,    # measure accum ready
        nc.semaphore("g1sem") as g1sem,
        nc.semaphore("cpsem") as cpsem,
        nc.semaphore("g2sem") as g2sem,
        nc.semaphore("fin") as fin,
        nc.semaphore("ssfin") as ssfin,
        nc.semaphore("outd") as outd,
        nc.semaphore("dsync") as dsync,
    ):

        @block.sync
        def _(sync):
            for src, dst in [
                (odd_d, ODD), (evpmfz_d, EVPMFZ), (fz_d, FZ), (liveb_d, LIVEB),
                (biasb_d, BIASB), (initm_d, INITM), (g1_d, G1S), (g2_d, G2S),
            ]:
                sync.dma_start(out=dst[:], in_=src[:]).then_inc(cdma, 16)
            for g in range(G + 2):
                if g < G:
                    if g >= 4:
                        sync.wait_ge(acts, g - 3)
                    r = r_of(g)
                    sync.dma_start(
                        out=XR[g % 4][0:r * NB, :],
                        in_=x_d[:, 8 * g:8 * g + r, :].rearrange("b t k -> t b k"),
                    ).then_inc(xdma[g % 4], 16)
                if g >= 2:
                    # ER8 load for tile g-2 (after its store; ring depth 3)
                    gl = g - 2
                    sync.wait_ge(emst, 16 * (gl + 1))
                    if gl >= 3:
                        sync.wait_ge(cons, gl - 2)
                    sync.dma_start(
                        out=ER8[gl % 3][:],
                        in_=emit_d[gl].rearrange("t b j w -> (b j) t w"),
                    ).then_inc(elo, 16)

        @block.scalar
        def _(scalar):
            scalar.wait_ge(cdma, 128)
            for g in range(G):
                scalar.wait_ge(xdma[g % 4], 16 * (g // 4 + 1))
                if g >= 4:
                    scalar.wait_ge(pa, g - 3)
                r = r_of(g)
                nc.scalar.activation(
                    out=ER4[g % 4][0:r * NB, 0:TK], in_=XR[g % 4][0:r * NB, :],
                    func=ACTF.Exp, bias=BIASB[0:r * NB, g:g + 1],
                    accum_out=SSD[0:r * NB, g:g + 1],
                ).then_inc(acts, 1)
                # even-state pattern, live-scaled (freeze handled by Pool +FZ)
                if g >= 3:
                    scalar.wait_ge(emst, 16 * (g - 2))
                nc.scalar.activation(
                    out=EMR[g % 3][0:r * NB, :, 10:TW:2], in_=EVPMFZ[0:r * NB],
                    func=ACTF.Copy, scale=LIVEB[0:r * NB, g:g + 1],
                ).then_inc(evc, 1)
                # issue store for tile g-1 (Pool done by then)
                if g >= 1:
                    scalar.wait_ge(pc, g)
                    rp = r_of(g - 1)
                    scalar.dma_start(
                        out=emit_d[g - 1][0:rp].rearrange("t b j w -> (t b) j w"),
                        in_=EMR[(g - 1) % 3][0:rp * NB],
                    ).then_inc(emst, 16)
            scalar.wait_ge(pc, G)
            rp = r_of(G - 1)
            scalar.dma_start(
                out=emit_d[G - 1][0:rp].rearrange("t b j w -> (t b) j w"),
                in_=EMR[(G - 1) % 3][0:rp * NB],
            ).then_inc(emst, 16)
            nc.scalar.activation(
                out=INV[:], in_=INV[:], func=ACTF.Copy).then_inc(ssfin, 1)

        @block.gpsimd
        def _(gpsimd):
            gpsimd.wait_ge(cdma, 128)
            for g in range(G):
                r = r_of(g)
                # B2: add freeze one-hot to the live-scaled even pattern
                gpsimd.wait_ge(evc, g + 1)
                nc.gpsimd.tensor_tensor(
                    out=EMR[g % 3][0:r * NB, :, 10:TW:2],
                    in0=EMR[g % 3][0:r * NB, :, 10:TW:2],
                    in1=FZ[0:r * NB], op=ALU.add)
                # A: odd-state emissions (dead rows killed via exp bias)
                gpsimd.wait_ge(acts, g + 1)
                nc.gpsimd.tensor_tensor(
                    out=EMR[g % 3][0:r * NB, :, 11:TW:2],
                    in0=ER4[g % 4][0:r * NB].rearrange("p (j q) -> p j q", j=NCH),
                    in1=ODD[0:r * NB], op=ALU.mult).then_inc(pa, 1)
                # C: duplicate chunk tails into next chunk's ghost cols
                nc.gpsimd.tensor_copy(
                    out=EMR[g % 3][0:r * NB, 1:NCH, 0:GW],
                    in_=EMR[g % 3][0:r * NB, 0:NCH - 1, W:TW],
                ).then_inc(pc, 1)
            # final exports
            gpsimd.wait_ge(fin, 1)
            gpsimd.dma_start(out=alpha_d[:], in_=AB[(Tmax - 1) % 2][:]).then_inc(outd, 16)
            gpsimd.dma_start(out=outsb_d[:], in_=OUTSB[:]).then_inc(outd, 16)
            gpsimd.wait_ge(ssfin, 1)
            gpsimd.dma_start(out=ss_d[:], in_=SSD[:]).then_inc(outd, 16)

        @block.tensor
        def _(tensor):
            tensor.wait_ge(cdma, 128)
            for m in range(NR):
                tensor.wait_ge(msem, m + 1)
                nc.tensor.matmul(PS1[:], G1S[:], MS[m % 2][:],
                                 start=True, stop=True).then_inc(g1sem, 1)
                tensor.wait_ge(cpsem, m + 1)
                nc.tensor.matmul(PS2[:], G2S[:], OUTSB[:, m:m + 1],
                                 start=True, stop=True).then_inc(g2sem, 1)

        @block.vector
        def _(vector):
            vector.wait_ge(cdma, 128)
            for buf in AB:
                nc.vector.memset(buf[:], 0.0)
            nc.vector.memset(INV[:], 1.0)
            vector.wait_ge(elo, 16)
            # alpha_0 = E_0 * INITM  (states 0,1 of chunk 0)
            nc.vector.tensor_tensor(
                out=AB[0][:], in0=ER8[0][:, 0, :], in1=INITM[:], op=ALU.mult)
            import os as _os
            DRM = int(_os.environ.get("BASS_DRAINS", "15"))
            SEMMODE = _os.environ.get("BASS_SYNCMODE", "drain") == "sem"
            nsync = [0]

            def bsync(prod, bit, force_drain=False):
                # RAW barrier after `prod` when bit enabled: drain, or
                # completion-semaphore self-wait.
                if not (DRM & bit):
                    return
                if SEMMODE and not force_drain:
                    nsync[0] += 1
                    prod.then_inc(dsync, 1)
                    vector.wait_ge(dsync, nsync[0])
                else:
                    nc.vector.drain()

            for t in range(1, Tmax):
                g, tl = t // 8, t % 8
                CUR, NXT = AB[(t - 1) % 2], AB[t % 2]
                if (t - 1) % KSH == 0:
                    nc.vector.stream_shuffle(
                        out=CUR[:, 0:GW], in_=CUR[:, W:TW], mask=SHMASK)
                    if DRM & 1:
                        nc.vector.drain()
                if tl == 0:
                    vector.wait_ge(elo, 16 * (g + 1))
                m_cp = cp_at.get(t)
                if m_cp is not None:
                    vector.wait_ge(g1sem, m_cp + 1)
                    nc.vector.tensor_copy(
                        out=OUTSB[:, m_cp:m_cp + 1], in_=PS1[:]).then_inc(cpsem, 1)
                m_ap = ap_at.get(t)
                if m_ap is not None:
                    vector.wait_ge(g2sem, m_ap + 1)
                    nc.vector.reciprocal(out=INV[:], in_=PS2[:])
                ta = nc.vector.tensor_add(
                    NXT[:, 2:TW], CUR[:, 2:TW], CUR[:, 1:TW - 1])
                bsync(ta, 2)
                fx = nc.vector.tensor_add(
                    NXT[:, 3:TW:2], NXT[:, 3:TW:2], CUR[:, 1:TW - 2:2])
                bsync(fx, 4)
                m_me = me_at.get(t)
                kw = {}
                if m_me is not None:
                    kw["accum_out"] = MS[m_me % 2][:]
                st = nc.vector.scalar_tensor_tensor(
                    out=NXT[:, 2:TW], in0=NXT[:, 2:TW],
                    scalar=(INV[:] if m_ap is not None else 1.0),
                    in1=ER8[g % 3][:, tl, 2:TW],
                    op0=ALU.mult, op1=ALU.mult, **kw)
                has_extra = (m_me is not None) or (tl == 7 or t == Tmax - 1)
                bsync(st, 8, force_drain=has_extra)
                if m_me is not None:
                    st.then_inc(msem, 1)
                if tl == 7 or t == Tmax - 1:
                    st.then_inc(cons, 1)
            nc.vector.memset(AB[(Tmax - 1) % 2][:, 0:1], 0.0).then_inc(fin, 1)

    stack.close()
    return nc


def _host_constants(in_lens_c, out_lens_c, Tmax, G):
    """Per-core constant tensors. Partition p = b*8 + j; stage-A rows (tl*16+b)."""
    b_row = np.arange(128) % NB             # stage-A row -> sample
    tl_row = np.arange(128) // NB           # stage-A row -> local t
    L = in_lens_c.astype(np.int64)          # [16]
    twoL = 2 * L
    # state grid per (j, q): odd states s = 52j + 2q + 1, even states s = 52j + 2q
    j_g, q_g = np.meshgrid(np.arange(NCH), np.arange(26), indexing="ij")
    s_odd = 52 * j_g + 2 * q_g + 1          # [8, 26]
    s_even = 52 * j_g + 2 * q_g
    odd = (s_odd[None] <= twoL[:, None, None]).astype(np.float32)       # [16,8,26]
    evp = EB * (s_even[None] <= twoL[:, None, None]).astype(np.float32)
    fz = (s_even[None] == twoL[:, None, None]).astype(np.float32)
    oddm = odd[b_row].astype(np.float32).copy()
    evpmfz = (evp - fz)[b_row].astype(np.float32).copy()
    fzp = fz[b_row].astype(np.float32).copy()
    # live[(tl*16+b), g] = t < ol_b with t = 8g + tl
    tt = 8 * np.arange(G)[None, :] + tl_row[:, None]                    # [128, G]
    liveb = (tt < out_lens_c[b_row][:, None]).astype(np.float32)
    biasb = ((liveb - 1.0) * 80.0).astype(np.float32)
    initm = np.zeros((128, TW), np.float32)
    p_j = np.arange(128) % NCH
    initm[(p_j == 0), GW] = 1.0
    initm[(p_j == 0), GW + 1] = 1.0
    p_b = np.arange(128) // NCH
    g1 = (p_b[:, None] == np.arange(NB)[None, :]).astype(np.float32)
    g2 = (np.arange(NB)[:, None] == p_b[None, :]).astype(np.float32)
    return dict(oddm=oddm, evpmfz=evpmfz, fzp=fzp, liveb=liveb, biasb=biasb,
                initm=initm, g1=g1, g2=g2)


def kernel(attn_logprob, in_lens, out_lens):
    x = np.ascontiguousarray(np.asarray(attn_logprob, np.float32)[:, 0])  # [128,900,200]
    il = np.asarray(in_lens).astype(np.int64)
    ol = np.asarray(out_lens).astype(np.int64)
    Bfull = x.shape[0]
    Tmax = int(ol.max())
    G = (Tmax + 7) // 8
    measures = [t for t in range(4, Tmax - 1 - LAG, RESC)]
    NR = len(measures)

    nc = _build(Tmax, measures, G, NR)

    in_maps = []
    for c in range(NCORES):
        sl = slice(c * NB, (c + 1) * NB)
        m = {"x": np.ascontiguousarray(x[sl])}
        m.update(_host_constants(il[sl], ol[sl], Tmax, G))
        in_maps.append(m)

    import os
    global LAST_RESULTS, LAST_EXEC_S
    LAST_EXEC_S = None
    LAST_RESULTS = run_bass_kernel_spmd(nc, in_maps, list(range(NCORES)))
    res = LAST_RESULTS.results
    if os.environ.get("BASS_PROFILE", "0") == "1":
        try:
            tdir = os.environ.get("BASS_TRACE_DIR") or None
            tr = run_bass_kernel_spmd(nc, in_maps, list(range(NCORES)),
                                      trace=True, tmpdir=tdir)
            if tr.exec_time_ns is not None:
                LAST_EXEC_S = tr.exec_time_ns * 1e-9
                LAST_RESULTS = tr
        except Exception as e:
            print("trace run failed:", e)
        if LAST_EXEC_S is None:
            import time
            ts = []
            for _ in range(3):
                t0 = time.time()
                run_bass_kernel_spmd(nc, in_maps, list(range(NCORES)))
                ts.append(time.time() - t0)
            LAST_EXEC_S = min(ts)

    # host reconstruction
    losses = []
    for c in range(NCORES):
        sl = slice(c * NB, (c + 1) * NB)
        alpha = np.asarray(res[c]["alpha"], np.float64)    # [128, TW]
        outsb = np.asarray(res[c]["outsb"], np.float64)    # [16, NR]
        ss = np.asarray(res[c]["ssden"], np.float64)       # [128, G]
        Lc = il[sl]
        olc = ol[sl]
        lb = np.zeros(NB)
        for b in range(NB):
            # denominators for frames t < ol_b
            ts_ = np.arange(olc[b])
            D = ss[(ts_ % 8) * NB + b, ts_ // 8] + EB
            lnD = np.sum(np.log(np.maximum(D, 1e-300)))
            lnM = np.sum(np.log(np.maximum(outsb[b, :NR], 1e-300)))
            s_hi = 2 * Lc[b]
            v = alpha[b * NCH + s_hi // W, GW + s_hi % W]
            if olc[b] == Tmax:
                s_lo = s_hi - 1
                v = v + alpha[b * NCH + s_lo // W, GW + s_lo % W]
            with np.errstate(divide="ignore", invalid="ignore"):
                ln_true = np.log(v) - lnD + lnM
            loss = -ln_true / Lc[b]
            if not np.isfinite(loss) or loss > 1e20:
                loss = 0.0
            lb[b] = loss
        losses.append(lb)
    return np.float32(np.mean(np.concatenate(losses)[:Bfull]))


# revision 29
# speedup vs baseline: 1.3605x; 1.3605x over previous
"""Bass/Trainium2 kernel for nn_AttentionCTCLoss (RAD-TTS attention CTC loss).

Pure data-parallel over 8 NeuronCores (16 samples each). Per core:
softmax over 201 classes kept UNNORMALIZED (raw exp; per-frame denominators
exported, folded out on host). The 900-step CTC alpha recursion runs in the
probability domain on a chunked layout: partitions p = b*8 + j, chunk j holds
states [52j, 52j+52) plus a 10-col ghost halo [52j-10, 52j) refreshed every 4
steps with a single DVE stream_shuffle (partition+1 copy within quadrants).
Per step the DVE does exactly 3 ops (add, strided skip-add, STT multiply by
emissions with rescale factor folded in). Per-sample rescale every 12 steps
via PE matmuls (measure accum -> G1 -> G2 -> reciprocal), applied with an
8-step lag. Final-state extraction uses an emission "freeze": for t >= ol_b
the emission row becomes one-hot 1.0 at state 2L_b, which makes
alpha[2L_b] = hi+lo and keeps it fixed, so no per-step snapshots are needed.
Host reconstructs loss_b = -(ln alpha[2L] - sum ln D_t + sum ln M_k) / L_b.
"""
import math
import numpy as np
from contextlib import ExitStack

import concourse.bass as bass
import concourse.mybir as mybir
from concourse.bass_utils import run_bass_kernel_spmd

LAST_RESULTS = None
LAST_EXEC_S = None

F32 = mybir.dt.float32
ALU = mybir.AluOpType
ACTF = mybir.ActivationFunctionType

NCORES = 8
NB = 16            # samples per core
TQ, TK = 900, 200
NCH = 8            # state chunks per sample
W = 52             # interior states per chunk (8*52 = 416 >= 401)
GW = 10            # ghost halo columns
TW = GW + W        # tile width 62
KSH = 4            # ghost refresh (stream_shuffle) period
RESC = 12          # rescale period
LAG = 8            # measure -> apply lag
EB = math.exp(-1.0)
SHMASK = [i - 1 if i % 8 else i + 7 for i in range(32)]
PADW = 104         # padded alpha tile width (junk cols >= 74 grow unbounded)
MULW = 72          # mult op width (cols 2:74), pads DVE pipe past RAW depth
ERW = 512          # ER8 flat width (8*62 data + 16 pad cols at 1.0)


def _build(Tmax, measures, G, NR):
    nc = bass.Bass()
    x_d = nc.declare_dram_parameter("x", [NB, TQ, TK], F32, isOutput=False)
    odd_d = nc.declare_dram_parameter("oddm", [128, NCH, 26], F32, isOutput=False)
    evpmfz_d = nc.declare_dram_parameter("evpmfz", [128, NCH, 26], F32, isOutput=False)
    fz_d = nc.declare_dram_parameter("fzp", [128, NCH, 26], F32, isOutput=False)
    liveb_d = nc.declare_dram_parameter("liveb", [128, G], F32, isOutput=False)
    biasb_d = nc.declare_dram_parameter("biasb", [128, G], F32, isOutput=False)
    initm_d = nc.declare_dram_parameter("initm", [128, TW], F32, isOutput=False)
    g1_d = nc.declare_dram_parameter("g1", [128, NB], F32, isOutput=False)
    g2_d = nc.declare_dram_parameter("g2", [NB, 128], F32, isOutput=False)
    alpha_d = nc.declare_dram_parameter("alpha", [128, TW], F32, isOutput=True)
    outsb_d = nc.declare_dram_parameter("outsb", [NB, max(NR, 1)], F32, isOutput=True)
    ss_d = nc.declare_dram_parameter("ssden", [128, G], F32, isOutput=True)
    emit_d = nc.dram_tensor("emitd", [G, 8, NB, NCH, TW], F32)

    stack = ExitStack()
    def sb(name, shape):
        return stack.enter_context(nc.sbuf_tensor(name, shape, F32))
    XR = [sb("xr%d" % i, [128, TK]) for i in range(4)]
    ER4 = [sb("er4_%d" % i, [128, NCH * 26]) for i in range(4)]
    EMR = [sb("emr%d" % i, [128, NCH, TW]) for i in range(3)]
    ER8 = [sb("er8_%d" % i, [128, ERW]) for i in range(3)]
    ODD = sb("oddsb", [128, NCH, 26])
    EVPMFZ = sb("evpmfzsb", [128, NCH, 26])
    FZ = sb("fzsb", [128, NCH, 26])
    LIVEB = sb("livebsb", [128, G])
    BIASB = sb("biasbsb", [128, G])
    INITM = sb("initmsb", [128, TW])
    G1S = sb("g1sb", [128, NB])
    G2S = sb("g2sb", [NB, 128])
    AB = [sb("ab%d" % i, [128, PADW]) for i in range(2)]
    MS = [sb("ms%d" % i, [128, 1]) for i in range(2)]
    INV = sb("invsb", [128, 1])
    OUTSB = sb("outsbsb", [NB, max(NR, 1)])
    SSD = sb("ssdsb", [128, G])
    PS1 = stack.enter_context(nc.psum_tensor("ps1t", [NB, 1], F32))
    PS2 = stack.enter_context(nc.psum_tensor("ps2t", [128, 1], F32))

    r_of = lambda g: min(8, Tmax - 8 * g)
    me_at = {t: m for m, t in enumerate(measures)}
    cp_at = {t + LAG // 2: m for m, t in enumerate(measures)}
    ap_at = {t + LAG: m for m, t in enumerate(measures)}

    xdma = [stack.enter_context(nc.semaphore("xdma%d" % i)) for i in range(4)]
    with (
        nc.Block() as block,
        nc.semaphore("cdma") as cdma,
        nc.semaphore("acts") as acts,
        nc.semaphore("evc") as evc,      # scalar even-pattern copy done
        nc.semaphore("pa") as pa,        # Pool finished A (ER4 consumed)
        nc.semaphore("pc") as pc,        # Pool finished tile build
        nc.semaphore("emst") as emst,    # emit store DMA complete
        nc.semaphore("elo") as elo,      # ER8 load complete
        nc.semaphore("cons") as cons,    # DVE consumed tile
        nc.semaphore("msem") as msem,    # measure accum ready
        nc.semaphore("g1sem") as g1sem,
        nc.semaphore("cpsem") as cpsem,
        nc.semaphore("g2sem") as g2sem,
        nc.semaphore("fin") as fin,
        nc.semaphore("ssfin") as ssfin,
        nc.semaphore("outd") as outd,
        nc.semaphore("dsync") as dsync,
    ):

        @block.sync
        def _(sync):
            for src, dst in [
                (odd_d, ODD), (evpmfz_d, EVPMFZ), (fz_d, FZ), (liveb_d, LIVEB),
                (biasb_d, BIASB), (initm_d, INITM), (g1_d, G1S), (g2_d, G2S),
            ]:
                sync.dma_start(out=dst[:], in_=src[:]).then_inc(cdma, 16)
            for g in range(G + 2):
                if g < G:
                    if g >= 4:
                        sync.wait_ge(acts, g - 3)
                    r = r_of(g)
                    sync.dma_start(
                        out=XR[g % 4][0:r * NB, :],
                        in_=x_d[:, 8 * g:8 * g + r, :].rearrange("b t k -> t b k"),
                    ).then_inc(xdma[g % 4], 16)
                if g >= 2:
                    # ER8 load for tile g-2 (after its store; ring depth 3)
                    gl = g - 2
                    sync.wait_ge(emst, 16 * (gl + 1))
                    if gl >= 3:
                        sync.wait_ge(cons, gl - 2)
                    sync.dma_start(
                        out=ER8[gl % 3][:, 0:8 * TW].rearrange(
                            "p (t w) -> p t w", t=8),
                        in_=emit_d[gl].rearrange("t b j w -> (b j) t w"),
                    ).then_inc(elo, 16)

        @block.scalar
        def _(scalar):
            scalar.wait_ge(cdma, 128)
            for g in range(G):
                scalar.wait_ge(xdma[g % 4], 16 * (g // 4 + 1))
                if g >= 4:
                    scalar.wait_ge(pa, g - 3)
                r = r_of(g)
                nc.scalar.activation(
                    out=ER4[g % 4][0:r * NB, 0:TK], in_=XR[g % 4][0:r * NB, :],
                    func=ACTF.Exp, bias=BIASB[0:r * NB, g:g + 1],
                    accum_out=SSD[0:r * NB, g:g + 1],
                ).then_inc(acts, 1)
                # even-state pattern, live-scaled (freeze handled by Pool +FZ)
                if g >= 3:
                    scalar.wait_ge(emst, 16 * (g - 2))
                nc.scalar.activation(
                    out=EMR[g % 3][0:r * NB, :, 10:TW:2], in_=EVPMFZ[0:r * NB],
                    func=ACTF.Copy, scale=LIVEB[0:r * NB, g:g + 1],
                ).then_inc(evc, 1)
                # issue store for tile g-1 (Pool done by then)
                if g >= 1:
                    scalar.wait_ge(pc, g)
                    rp = r_of(g - 1)
                    scalar.dma_start(
                        out=emit_d[g - 1][0:rp].rearrange("t b j w -> (t b) j w"),
                        in_=EMR[(g - 1) % 3][0:rp * NB],
                    ).then_inc(emst, 16)
            scalar.wait_ge(pc, G)
            rp = r_of(G - 1)
            scalar.dma_start(
                out=emit_d[G - 1][0:rp].rearrange("t b j w -> (t b) j w"),
                in_=EMR[(G - 1) % 3][0:rp * NB],
            ).then_inc(emst, 16)
            nc.scalar.activation(
                out=INV[:], in_=INV[:], func=ACTF.Copy).then_inc(ssfin, 1)

        @block.gpsimd
        def _(gpsimd):
            gpsimd.wait_ge(cdma, 128)
            for g in range(G):
                r = r_of(g)
                # B2: add freeze one-hot to the live-scaled even pattern
                gpsimd.wait_ge(evc, g + 1)
                nc.gpsimd.tensor_tensor(
                    out=EMR[g % 3][0:r * NB, :, 10:TW:2],
                    in0=EMR[g % 3][0:r * NB, :, 10:TW:2],
                    in1=FZ[0:r * NB], op=ALU.add)
                # A: odd-state emissions (dead rows killed via exp bias)
                gpsimd.wait_ge(acts, g + 1)
                nc.gpsimd.tensor_tensor(
                    out=EMR[g % 3][0:r * NB, :, 11:TW:2],
                    in0=ER4[g % 4][0:r * NB].rearrange("p (j q) -> p j q", j=NCH),
                    in1=ODD[0:r * NB], op=ALU.mult).then_inc(pa, 1)
                # C: duplicate chunk tails into next chunk's ghost cols
                nc.gpsimd.tensor_copy(
                    out=EMR[g % 3][0:r * NB, 1:NCH, 0:GW],
                    in_=EMR[g % 3][0:r * NB, 0:NCH - 1, W:TW],
                ).then_inc(pc, 1)
            # final exports
            gpsimd.wait_ge(fin, 1)
            gpsimd.dma_start(out=alpha_d[:], in_=AB[(Tmax - 1) % 2][:, 0:TW]).then_inc(outd, 16)
            gpsimd.dma_start(out=outsb_d[:], in_=OUTSB[:]).then_inc(outd, 16)
            gpsimd.wait_ge(ssfin, 1)
            gpsimd.dma_start(out=ss_d[:], in_=SSD[:]).then_inc(outd, 16)

        @block.tensor
        def _(tensor):
            tensor.wait_ge(cdma, 128)
            for m in range(NR):
                tensor.wait_ge(msem, m + 1)
                nc.tensor.matmul(PS1[:], G1S[:], MS[m % 2][:],
                                 start=True, stop=True).then_inc(g1sem, 1)
                tensor.wait_ge(cpsem, m + 1)
                nc.tensor.matmul(PS2[:], G2S[:], OUTSB[:, m:m + 1],
                                 start=True, stop=True).then_inc(g2sem, 1)

        @block.vector
        def _(vector):
            vector.wait_ge(cdma, 128)
            for buf in AB:
                nc.vector.memset(buf[:], 0.0)
            nc.vector.memset(INV[:], 1.0)
            for er in ER8:
                nc.vector.memset(er[:, 8 * TW:ERW], 1.0)
            vector.wait_ge(elo, 16)
            # alpha_0 = E_0 * INITM  (states 0,1 of chunk 0)
            nc.vector.tensor_tensor(
                out=AB[0][:, 0:TW], in0=ER8[0][:, 0:TW], in1=INITM[:],
                op=ALU.mult)
            # Pad-mode main loop: wide ops provide in-pipe RAW spacing
            # (write-visibility depth ~66 DVE cycles). Only fix->mult and
            # the shuffle boundaries need explicit drains.
            for t in range(1, Tmax):
                g, tl = t // 8, t % 8
                CUR, NXT = AB[(t - 1) % 2], AB[t % 2]
                if (t - 1) % KSH == 0:
                    nc.vector.drain()
                    nc.vector.stream_shuffle(
                        out=CUR[:, 0:GW], in_=CUR[:, W:TW], mask=SHMASK)
                    nc.vector.drain()
                if tl == 0:
                    vector.wait_ge(elo, 16 * (g + 1))
                m_cp = cp_at.get(t)
                if m_cp is not None:
                    vector.wait_ge(g1sem, m_cp + 1)
                    nc.vector.tensor_copy(
                        out=OUTSB[:, m_cp:m_cp + 1], in_=PS1[:]).then_inc(cpsem, 1)
                m_ap = ap_at.get(t)
                if m_ap is not None:
                    vector.wait_ge(g2sem, m_ap + 1)
                    nc.vector.reciprocal(out=INV[:], in_=PS2[:])
                nc.vector.tensor_add(
                    NXT[:, 2:PADW], CUR[:, 2:PADW], CUR[:, 1:PADW - 1])
                nc.vector.tensor_add(
                    NXT[:, 3:TW:2], NXT[:, 3:TW:2], CUR[:, 1:TW - 2:2])
                nc.vector.drain()
                m_me = me_at.get(t)
                if m_me is not None:
                    st = nc.vector.scalar_tensor_tensor(
                        out=NXT[:, 2:TW], in0=NXT[:, 2:TW],
                        scalar=(INV[:] if m_ap is not None else 1.0),
                        in1=ER8[g % 3][:, tl * TW + 2:tl * TW + TW],
                        op0=ALU.mult, op1=ALU.mult, accum_out=MS[m_me % 2][:])
                    nc.vector.drain()
                    st.then_inc(msem, 1)
                else:
                    st = nc.vector.scalar_tensor_tensor(
                        out=NXT[:, 2:2 + MULW], in0=NXT[:, 2:2 + MULW],
                        scalar=(INV[:] if m_ap is not None else 1.0),
                        in1=ER8[g % 3][:, tl * TW + 2:tl * TW + 2 + MULW],
                        op0=ALU.mult, op1=ALU.mult)
                if tl == 7 or t == Tmax - 1:
                    st.then_inc(cons, 1)
            nc.vector.memset(AB[(Tmax - 1) % 2][:, 0:1], 0.0).then_inc(fin, 1)

    stack.close()
    return nc


def _host_constants(in_lens_c, out_lens_c, Tmax, G):
    """Per-core constant tensors. Partition p = b*8 + j; stage-A rows (tl*16+b)."""
    b_row = np.arange(128) % NB             # stage-A row -> sample
    tl_row = np.arange(128) // NB           # stage-A row -> local t
    L = in_lens_c.astype(np.int64)          # [16]
    twoL = 2 * L
    # state grid per (j, q): odd states s = 52j + 2q + 1, even states s = 52j + 2q
    j_g, q_g = np.meshgrid(np.arange(NCH), np.arange(26), indexing="ij")
    s_odd = 52 * j_g + 2 * q_g + 1          # [8, 26]
    s_even = 52 * j_g + 2 * q_g
    odd = (s_odd[None] <= twoL[:, None, None]).astype(np.float32)       # [16,8,26]
    evp = EB * (s_even[None] <= twoL[:, None, None]).astype(np.float32)
    fz = (s_even[None] == twoL[:, None, None]).astype(np.float32)
    oddm = odd[b_row].astype(np.float32).copy()
    evpmfz = (evp - fz)[b_row].astype(np.float32).copy()
    fzp = fz[b_row].astype(np.float32).copy()
    # live[(tl*16+b), g] = t < ol_b with t = 8g + tl
    tt = 8 * np.arange(G)[None, :] + tl_row[:, None]                    # [128, G]
    liveb = (tt < out_lens_c[b_row][:, None]).astype(np.float32)
    biasb = ((liveb - 1.0) * 80.0).astype(np.float32)
    initm = np.zeros((128, TW), np.float32)
    p_j = np.arange(128) % NCH
    initm[(p_j == 0), GW] = 1.0
    initm[(p_j == 0), GW + 1] = 1.0
    p_b = np.arange(128) // NCH
    g1 = (p_b[:, None] == np.arange(NB)[None, :]).astype(np.float32)
    g2 = (np.arange(NB)[:, None] == p_b[None, :]).astype(np.float32)
    return dict(oddm=oddm, evpmfz=evpmfz, fzp=fzp, liveb=liveb, biasb=biasb,
                initm=initm, g1=g1, g2=g2)


def kernel(attn_logprob, in_lens, out_lens):
    x = np.ascontiguousarray(np.asarray(attn_logprob, np.float32)[:, 0])  # [128,900,200]
    il = np.asarray(in_lens).astype(np.int64)
    ol = np.asarray(out_lens).astype(np.int64)
    Bfull = x.shape[0]
    Tmax = int(ol.max())
    G = (Tmax + 7) // 8
    measures = [t for t in range(4, Tmax - 1 - LAG, RESC)]
    NR = len(measures)

    nc = _build(Tmax, measures, G, NR)

    in_maps = []
    for c in range(NCORES):
        sl = slice(c * NB, (c + 1) * NB)
        m = {"x": np.ascontiguousarray(x[sl])}
        m.update(_host_constants(il[sl], ol[sl], Tmax, G))
        in_maps.append(m)

    import os
    global LAST_RESULTS, LAST_EXEC_S
    LAST_EXEC_S = None
    LAST_RESULTS = run_bass_kernel_spmd(nc, in_maps, list(range(NCORES)))
    res = LAST_RESULTS.results
    if os.environ.get("BASS_PROFILE", "0") == "1":
        try:
            tdir = os.environ.get("BASS_TRACE_DIR") or None
            tr = run_bass_kernel_spmd(nc, in_maps, list(range(NCORES)),
                                      trace=True, tmpdir=tdir)
            if tr.exec_time_ns is not None:
                LAST_EXEC_S = tr.exec_time_ns * 1e-9
                LAST_RESULTS = tr
        except Exception as e:
            print("trace run failed:", e)
        if LAST_EXEC_S is None:
            import time
            ts = []
            for _ in range(3):
                t0 = time.time()
                run_bass_kernel_spmd(nc, in_maps, list(range(NCORES)))
                ts.append(time.time() - t0)
            LAST_EXEC_S = min(ts)

    # host reconstruction
    losses = []
    for c in range(NCORES):
        sl = slice(c * NB, (c + 1) * NB)
        alpha = np.asarray(res[c]["alpha"], np.float64)    # [128, TW]
        outsb = np.asarray(res[c]["outsb"], np.float64)    # [16, NR]
        ss = np.asarray(res[c]["ssden"], np.float64)       # [128, G]
        Lc = il[sl]
        olc = ol[sl]
        lb = np.zeros(NB)
        for b in range(NB):
            # denominators for frames t < ol_b
            ts_ = np.arange(olc[b])
            D = ss[(ts_ % 8) * NB + b, ts_ // 8] + EB
            lnD = np.sum(np.log(np.maximum(D, 1e-300)))
            lnM = np.sum(np.log(np.maximum(outsb[b, :NR], 1e-300)))
            s_hi = 2 * Lc[b]
            v = alpha[b * NCH + s_hi // W, GW + s_hi % W]
            if olc[b] == Tmax:
                s_lo = s_hi - 1
                v = v + alpha[b * NCH + s_lo // W, GW + s_lo % W]
            with np.errstate(divide="ignore", invalid="ignore"):
                ln_true = np.log(v) - lnD + lnM
            loss = -ln_true / Lc[b]
            if not np.isfinite(loss) or loss > 1e20:
                loss = 0.0
            lb[b] = loss
        losses.append(lb)
    return np.float32(np.mean(np.concatenate(losses)[:Bfull]))


# revision 30
# speedup vs baseline: 1.6096x; 1.1831x over previous
"""Bass/Trainium2 kernel for nn_AttentionCTCLoss (RAD-TTS attention CTC loss).

Pure data-parallel over 8 NeuronCores (16 samples each). Per core:
softmax over 201 classes kept UNNORMALIZED (raw exp; per-frame denominators
exported, folded out on host). The 900-step CTC alpha recursion runs in the
probability domain on a chunked layout: partitions p = b*8 + j, chunk j holds
states [52j, 52j+52) plus a 10-col ghost halo [52j-10, 52j) refreshed every 4
steps with a single DVE stream_shuffle (partition+1 copy within quadrants).
Per step the DVE does exactly 3 ops (add, strided skip-add, STT multiply by
emissions with rescale factor folded in). Per-sample rescale every 12 steps
via PE matmuls (measure accum -> G1 -> G2 -> reciprocal), applied with an
8-step lag. Final-state extraction uses an emission "freeze": for t >= ol_b
the emission row becomes one-hot 1.0 at state 2L_b, which makes
alpha[2L_b] = hi+lo and keeps it fixed, so no per-step snapshots are needed.
Host reconstructs loss_b = -(ln alpha[2L] - sum ln D_t + sum ln M_k) / L_b.
"""
import math
import numpy as np
from contextlib import ExitStack

import concourse.bass as bass
import concourse.mybir as mybir
from concourse.bass_utils import run_bass_kernel_spmd

LAST_RESULTS = None
LAST_EXEC_S = None

F32 = mybir.dt.float32
ALU = mybir.AluOpType
ACTF = mybir.ActivationFunctionType

NCORES = 8
NB = 16            # samples per core
TQ, TK = 900, 200
NCH = 8            # state chunks per sample
W = 52             # interior states per chunk (8*52 = 416 >= 401)
GW = 10            # ghost halo columns
TW = GW + W        # tile width 62
KSH = 4            # ghost refresh (stream_shuffle) period
RESC = 12          # rescale period
LAG = 8            # measure -> apply lag
EB = math.exp(-1.0)
SHMASK = [i - 1 if i % 8 else i + 7 for i in range(32)]
PADW = 140         # padded alpha tile width (junk cols >= 74 grow unbounded)
MULW = 72          # mult op width (cols 2:74), pads DVE pipe past RAW depth
ERW = 512          # ER8 flat width (8*62 data + 16 pad cols at 1.0)


def _build(Tmax, measures, G, NR):
    nc = bass.Bass()
    x_d = nc.declare_dram_parameter("x", [NB, TQ, TK], F32, isOutput=False)
    odd_d = nc.declare_dram_parameter("oddm", [128, NCH, 26], F32, isOutput=False)
    evpmfz_d = nc.declare_dram_parameter("evpmfz", [128, NCH, 26], F32, isOutput=False)
    fz_d = nc.declare_dram_parameter("fzp", [128, NCH, 26], F32, isOutput=False)
    liveb_d = nc.declare_dram_parameter("liveb", [128, G], F32, isOutput=False)
    biasb_d = nc.declare_dram_parameter("biasb", [128, G], F32, isOutput=False)
    initm_d = nc.declare_dram_parameter("initm", [128, TW], F32, isOutput=False)
    g1_d = nc.declare_dram_parameter("g1", [128, NB], F32, isOutput=False)
    g2_d = nc.declare_dram_parameter("g2", [NB, 128], F32, isOutput=False)
    alpha_d = nc.declare_dram_parameter("alpha", [128, TW], F32, isOutput=True)
    outsb_d = nc.declare_dram_parameter("outsb", [NB, max(NR, 1)], F32, isOutput=True)
    ss_d = nc.declare_dram_parameter("ssden", [128, G], F32, isOutput=True)
    emit_d = nc.dram_tensor("emitd", [G, 8, NB, NCH, TW], F32)

    stack = ExitStack()
    def sb(name, shape):
        return stack.enter_context(nc.sbuf_tensor(name, shape, F32))
    XR = [sb("xr%d" % i, [128, TK]) for i in range(4)]
    ER4 = [sb("er4_%d" % i, [128, NCH * 26]) for i in range(4)]
    EMR = [sb("emr%d" % i, [128, NCH, TW]) for i in range(3)]
    ER8 = [sb("er8_%d" % i, [128, ERW]) for i in range(3)]
    ODD = sb("oddsb", [128, NCH, 26])
    EVPMFZ = sb("evpmfzsb", [128, NCH, 26])
    FZ = sb("fzsb", [128, NCH, 26])
    LIVEB = sb("livebsb", [128, G])
    BIASB = sb("biasbsb", [128, G])
    INITM = sb("initmsb", [128, TW])
    G1S = sb("g1sb", [128, NB])
    G2S = sb("g2sb", [NB, 128])
    AB = [sb("ab%d" % i, [128, PADW]) for i in range(2)]
    MS = [sb("ms%d" % i, [128, 1]) for i in range(2)]
    INV = sb("invsb", [128, 1])
    OUTSB = sb("outsbsb", [NB, max(NR, 1)])
    SSD = sb("ssdsb", [128, G])
    PS1 = stack.enter_context(nc.psum_tensor("ps1t", [NB, 1], F32))
    PS2 = stack.enter_context(nc.psum_tensor("ps2t", [128, 1], F32))

    r_of = lambda g: min(8, Tmax - 8 * g)
    me_at = {t: m for m, t in enumerate(measures)}
    cp_at = {t + LAG // 2: m for m, t in enumerate(measures)}
    ap_at = {t + LAG: m for m, t in enumerate(measures)}

    xdma = [stack.enter_context(nc.semaphore("xdma%d" % i)) for i in range(4)]
    with (
        nc.Block() as block,
        nc.semaphore("cdma") as cdma,
        nc.semaphore("acts") as acts,
        nc.semaphore("evc") as evc,      # scalar even-pattern copy done
        nc.semaphore("pa") as pa,        # Pool finished A (ER4 consumed)
        nc.semaphore("pc") as pc,        # Pool finished tile build
        nc.semaphore("emst") as emst,    # emit store DMA complete
        nc.semaphore("elo") as elo,      # ER8 load complete
        nc.semaphore("cons") as cons,    # DVE consumed tile
        nc.semaphore("msem") as msem,    # measure accum ready
        nc.semaphore("g1sem") as g1sem,
        nc.semaphore("cpsem") as cpsem,
        nc.semaphore("g2sem") as g2sem,
        nc.semaphore("fin") as fin,
        nc.semaphore("ssfin") as ssfin,
        nc.semaphore("outd") as outd,
        nc.semaphore("dsync") as dsync,
    ):

        @block.sync
        def _(sync):
            for src, dst in [
                (odd_d, ODD), (evpmfz_d, EVPMFZ), (fz_d, FZ), (liveb_d, LIVEB),
                (biasb_d, BIASB), (initm_d, INITM), (g1_d, G1S), (g2_d, G2S),
            ]:
                sync.dma_start(out=dst[:], in_=src[:]).then_inc(cdma, 16)
            for g in range(G + 2):
                if g < G:
                    if g >= 4:
                        sync.wait_ge(acts, g - 3)
                    r = r_of(g)
                    sync.dma_start(
                        out=XR[g % 4][0:r * NB, :],
                        in_=x_d[:, 8 * g:8 * g + r, :].rearrange("b t k -> t b k"),
                    ).then_inc(xdma[g % 4], 16)
                if g >= 2:
                    # ER8 load for tile g-2 (after its store; ring depth 3)
                    gl = g - 2
                    sync.wait_ge(emst, 16 * (gl + 1))
                    if gl >= 3:
                        sync.wait_ge(cons, gl - 2)
                    sync.dma_start(
                        out=ER8[gl % 3][:, 0:8 * TW].rearrange(
                            "p (t w) -> p t w", t=8),
                        in_=emit_d[gl].rearrange("t b j w -> (b j) t w"),
                    ).then_inc(elo, 16)

        @block.scalar
        def _(scalar):
            scalar.wait_ge(cdma, 128)
            for g in range(G):
                scalar.wait_ge(xdma[g % 4], 16 * (g // 4 + 1))
                if g >= 4:
                    scalar.wait_ge(pa, g - 3)
                r = r_of(g)
                nc.scalar.activation(
                    out=ER4[g % 4][0:r * NB, 0:TK], in_=XR[g % 4][0:r * NB, :],
                    func=ACTF.Exp, bias=BIASB[0:r * NB, g:g + 1],
                    accum_out=SSD[0:r * NB, g:g + 1],
                ).then_inc(acts, 1)
                # even-state pattern, live-scaled (freeze handled by Pool +FZ)
                if g >= 3:
                    scalar.wait_ge(emst, 16 * (g - 2))
                nc.scalar.activation(
                    out=EMR[g % 3][0:r * NB, :, 10:TW:2], in_=EVPMFZ[0:r * NB],
                    func=ACTF.Copy, scale=LIVEB[0:r * NB, g:g + 1],
                ).then_inc(evc, 1)
                # issue store for tile g-1 (Pool done by then)
                if g >= 1:
                    scalar.wait_ge(pc, g)
                    rp = r_of(g - 1)
                    scalar.dma_start(
                        out=emit_d[g - 1][0:rp].rearrange("t b j w -> (t b) j w"),
                        in_=EMR[(g - 1) % 3][0:rp * NB],
                    ).then_inc(emst, 16)
            scalar.wait_ge(pc, G)
            rp = r_of(G - 1)
            scalar.dma_start(
                out=emit_d[G - 1][0:rp].rearrange("t b j w -> (t b) j w"),
                in_=EMR[(G - 1) % 3][0:rp * NB],
            ).then_inc(emst, 16)
            nc.scalar.activation(
                out=INV[:], in_=INV[:], func=ACTF.Copy).then_inc(ssfin, 1)

        @block.gpsimd
        def _(gpsimd):
            gpsimd.wait_ge(cdma, 128)
            for g in range(G):
                r = r_of(g)
                # B2: add freeze one-hot to the live-scaled even pattern
                gpsimd.wait_ge(evc, g + 1)
                nc.gpsimd.tensor_tensor(
                    out=EMR[g % 3][0:r * NB, :, 10:TW:2],
                    in0=EMR[g % 3][0:r * NB, :, 10:TW:2],
                    in1=FZ[0:r * NB], op=ALU.add)
                # A: odd-state emissions (dead rows killed via exp bias)
                gpsimd.wait_ge(acts, g + 1)
                nc.gpsimd.tensor_tensor(
                    out=EMR[g % 3][0:r * NB, :, 11:TW:2],
                    in0=ER4[g % 4][0:r * NB].rearrange("p (j q) -> p j q", j=NCH),
                    in1=ODD[0:r * NB], op=ALU.mult).then_inc(pa, 1)
                # C: duplicate chunk tails into next chunk's ghost cols
                nc.gpsimd.tensor_copy(
                    out=EMR[g % 3][0:r * NB, 1:NCH, 0:GW],
                    in_=EMR[g % 3][0:r * NB, 0:NCH - 1, W:TW],
                ).then_inc(pc, 1)
            # final exports
            gpsimd.wait_ge(fin, 1)
            gpsimd.dma_start(out=alpha_d[:], in_=AB[(Tmax - 1) % 2][:, 0:TW]).then_inc(outd, 16)
            gpsimd.dma_start(out=outsb_d[:], in_=OUTSB[:]).then_inc(outd, 16)
            gpsimd.wait_ge(ssfin, 1)
            gpsimd.dma_start(out=ss_d[:], in_=SSD[:]).then_inc(outd, 16)

        @block.tensor
        def _(tensor):
            tensor.wait_ge(cdma, 128)
            for m in range(NR):
                tensor.wait_ge(msem, m + 1)
                nc.tensor.matmul(PS1[:], G1S[:], MS[m % 2][:],
                                 start=True, stop=True).then_inc(g1sem, 1)
                tensor.wait_ge(cpsem, m + 1)
                nc.tensor.matmul(PS2[:], G2S[:], OUTSB[:, m:m + 1],
                                 start=True, stop=True).then_inc(g2sem, 1)

        @block.vector
        def _(vector):
            vector.wait_ge(cdma, 128)
            for buf in AB:
                nc.vector.memset(buf[:], 0.0)
            nc.vector.memset(INV[:], 1.0)
            for er in ER8:
                nc.vector.memset(er[:, 8 * TW:ERW], 1.0)
            vector.wait_ge(elo, 16)
            # alpha_0 = E_0 * INITM  (states 0,1 of chunk 0)
            nc.vector.tensor_tensor(
                out=AB[0][:, 0:TW], in0=ER8[0][:, 0:TW], in1=INITM[:],
                op=ALU.mult)
            # Pad-mode main loop: wide ops provide in-pipe RAW spacing
            # (write-visibility depth ~66 DVE cycles). Only fix->mult and
            # the shuffle boundaries need explicit drains.
            for t in range(1, Tmax):
                g, tl = t // 8, t % 8
                CUR, NXT = AB[(t - 1) % 2], AB[t % 2]
                if (t - 1) % KSH == 0:
                    nc.vector.drain()
                    nc.vector.stream_shuffle(
                        out=CUR[:, 0:GW], in_=CUR[:, W:TW], mask=SHMASK)
                    nc.vector.drain()
                if tl == 0:
                    vector.wait_ge(elo, 16 * (g + 1))
                m_cp = cp_at.get(t)
                if m_cp is not None:
                    vector.wait_ge(g1sem, m_cp + 1)
                    nc.vector.tensor_copy(
                        out=OUTSB[:, m_cp:m_cp + 1], in_=PS1[:]).then_inc(cpsem, 1)
                m_ap = ap_at.get(t)
                if m_ap is not None:
                    vector.wait_ge(g2sem, m_ap + 1)
                    nc.vector.reciprocal(out=INV[:], in_=PS2[:])
                nc.vector.tensor_add(
                    NXT[:, 2:PADW], CUR[:, 2:PADW], CUR[:, 1:PADW - 1])
                nc.vector.tensor_add(
                    NXT[:, 3:PADW - 1:2], NXT[:, 3:PADW - 1:2],
                    CUR[:, 1:PADW - 3:2])
                m_me = me_at.get(t)
                if m_me is not None:
                    st = nc.vector.scalar_tensor_tensor(
                        out=NXT[:, 2:TW], in0=NXT[:, 2:TW],
                        scalar=(INV[:] if m_ap is not None else 1.0),
                        in1=ER8[g % 3][:, tl * TW + 2:tl * TW + TW],
                        op0=ALU.mult, op1=ALU.mult, accum_out=MS[m_me % 2][:])
                    nc.vector.drain()
                    st.then_inc(msem, 1)
                else:
                    st = nc.vector.scalar_tensor_tensor(
                        out=NXT[:, 2:2 + MULW], in0=NXT[:, 2:2 + MULW],
                        scalar=(INV[:] if m_ap is not None else 1.0),
                        in1=ER8[g % 3][:, tl * TW + 2:tl * TW + 2 + MULW],
                        op0=ALU.mult, op1=ALU.mult)
                if tl == 7 or t == Tmax - 1:
                    st.then_inc(cons, 1)
            nc.vector.memset(AB[(Tmax - 1) % 2][:, 0:1], 0.0).then_inc(fin, 1)

    stack.close()
    return nc


def _host_constants(in_lens_c, out_lens_c, Tmax, G):
    """Per-core constant tensors. Partition p = b*8 + j; stage-A rows (tl*16+b)."""
    b_row = np.arange(128) % NB             # stage-A row -> sample
    tl_row = np.arange(128) // NB           # stage-A row -> local t
    L = in_lens_c.astype(np.int64)          # [16]
    twoL = 2 * L
    # state grid per (j, q): odd states s = 52j + 2q + 1, even states s = 52j + 2q
    j_g, q_g = np.meshgrid(np.arange(NCH), np.arange(26), indexing="ij")
    s_odd = 52 * j_g + 2 * q_g + 1          # [8, 26]
    s_even = 52 * j_g + 2 * q_g
    odd = (s_odd[None] <= twoL[:, None, None]).astype(np.float32)       # [16,8,26]
    evp = EB * (s_even[None] <= twoL[:, None, None]).astype(np.float32)
    fz = (s_even[None] == twoL[:, None, None]).astype(np.float32)
    oddm = odd[b_row].astype(np.float32).copy()
    evpmfz = (evp - fz)[b_row].astype(np.float32).copy()
    fzp = fz[b_row].astype(np.float32).copy()
    # live[(tl*16+b), g] = t < ol_b with t = 8g + tl
    tt = 8 * np.arange(G)[None, :] + tl_row[:, None]                    # [128, G]
    liveb = (tt < out_lens_c[b_row][:, None]).astype(np.float32)
    biasb = ((liveb - 1.0) * 80.0).astype(np.float32)
    initm = np.zeros((128, TW), np.float32)
    p_j = np.arange(128) % NCH
    initm[(p_j == 0), GW] = 1.0
    initm[(p_j == 0), GW + 1] = 1.0
    p_b = np.arange(128) // NCH
    g1 = (p_b[:, None] == np.arange(NB)[None, :]).astype(np.float32)
    g2 = (np.arange(NB)[:, None] == p_b[None, :]).astype(np.float32)
    return dict(oddm=oddm, evpmfz=evpmfz, fzp=fzp, liveb=liveb, biasb=biasb,
                initm=initm, g1=g1, g2=g2)


def kernel(attn_logprob, in_lens, out_lens):
    x = np.ascontiguousarray(np.asarray(attn_logprob, np.float32)[:, 0])  # [128,900,200]
    il = np.asarray(in_lens).astype(np.int64)
    ol = np.asarray(out_lens).astype(np.int64)
    Bfull = x.shape[0]
    Tmax = int(ol.max())
    G = (Tmax + 7) // 8
    measures = [t for t in range(4, Tmax - 1 - LAG, RESC)]
    NR = len(measures)

    nc = _build(Tmax, measures, G, NR)

    in_maps = []
    for c in range(NCORES):
        sl = slice(c * NB, (c + 1) * NB)
        m = {"x": np.ascontiguousarray(x[sl])}
        m.update(_host_constants(il[sl], ol[sl], Tmax, G))
        in_maps.append(m)

    import os
    global LAST_RESULTS, LAST_EXEC_S
    LAST_EXEC_S = None
    LAST_RESULTS = run_bass_kernel_spmd(nc, in_maps, list(range(NCORES)))
    res = LAST_RESULTS.results
    if os.environ.get("BASS_PROFILE", "0") == "1":
        try:
            tdir = os.environ.get("BASS_TRACE_DIR") or None
            tr = run_bass_kernel_spmd(nc, in_maps, list(range(NCORES)),
                                      trace=True, tmpdir=tdir)
            if tr.exec_time_ns is not None:
                LAST_EXEC_S = tr.exec_time_ns * 1e-9
                LAST_RESULTS = tr
        except Exception as e:
            print("trace run failed:", e)
        if LAST_EXEC_S is None:
            import time
            ts = []
            for _ in range(3):
                t0 = time.time()
                run_bass_kernel_spmd(nc, in_maps, list(range(NCORES)))
                ts.append(time.time() - t0)
            LAST_EXEC_S = min(ts)

    # host reconstruction
    losses = []
    for c in range(NCORES):
        sl = slice(c * NB, (c + 1) * NB)
        alpha = np.asarray(res[c]["alpha"], np.float64)    # [128, TW]
        outsb = np.asarray(res[c]["outsb"], np.float64)    # [16, NR]
        ss = np.asarray(res[c]["ssden"], np.float64)       # [128, G]
        Lc = il[sl]
        olc = ol[sl]
        lb = np.zeros(NB)
        for b in range(NB):
            # denominators for frames t < ol_b
            ts_ = np.arange(olc[b])
            D = ss[(ts_ % 8) * NB + b, ts_ // 8] + EB
            lnD = np.sum(np.log(np.maximum(D, 1e-300)))
            lnM = np.sum(np.log(np.maximum(outsb[b, :NR], 1e-300)))
            s_hi = 2 * Lc[b]
            v = alpha[b * NCH + s_hi // W, GW + s_hi % W]
            if olc[b] == Tmax:
                s_lo = s_hi - 1
                v = v + alpha[b * NCH + s_lo // W, GW + s_lo % W]
            with np.errstate(divide="ignore", invalid="ignore"):
                ln_true = np.log(v) - lnD + lnM
            loss = -ln_true / Lc[b]
            if not np.isfinite(loss) or loss > 1e20:
                loss = 0.0
            lb[b] = loss
        losses.append(lb)
    return np.float32(np.mean(np.concatenate(losses)[:Bfull]))


# revision 31
# speedup vs baseline: 1.7196x; 1.0684x over previous
"""Bass/Trainium2 kernel for nn_AttentionCTCLoss (RAD-TTS attention CTC loss).

Pure data-parallel over 8 NeuronCores (16 samples each). Per core:
softmax over 201 classes kept UNNORMALIZED (raw exp; per-frame denominators
exported, folded out on host). The 900-step CTC alpha recursion runs in the
probability domain on a chunked layout: partitions p = b*8 + j, chunk j holds
states [52j, 52j+52) plus a 10-col ghost halo [52j-10, 52j) refreshed every 4
steps with a single DVE stream_shuffle (partition+1 copy within quadrants).
Per step the DVE does exactly 3 ops (add, strided skip-add, STT multiply by
emissions with rescale factor folded in). Per-sample rescale every 12 steps
via PE matmuls (measure accum -> G1 -> G2 -> reciprocal), applied with an
8-step lag. Final-state extraction uses an emission "freeze": for t >= ol_b
the emission row becomes one-hot 1.0 at state 2L_b, which makes
alpha[2L_b] = hi+lo and keeps it fixed, so no per-step snapshots are needed.
Host reconstructs loss_b = -(ln alpha[2L] - sum ln D_t + sum ln M_k) / L_b.
"""
import math
import numpy as np
from contextlib import ExitStack

import concourse.bass as bass
import concourse.mybir as mybir
from concourse.bass_utils import run_bass_kernel_spmd

LAST_RESULTS = None
LAST_EXEC_S = None

F32 = mybir.dt.float32
ALU = mybir.AluOpType
ACTF = mybir.ActivationFunctionType

NCORES = 8
NB = 16            # samples per core
TQ, TK = 900, 200
NCH = 8            # state chunks per sample
W = 52             # interior states per chunk (8*52 = 416 >= 401)
GW = 18            # ghost halo columns
TW = GW + W        # tile width 62
KSH = 8            # ghost refresh (stream_shuffle) period
RESC = 12          # rescale period
LAG = 8            # measure -> apply lag
EB = math.exp(-1.0)
SHMASK = [i - 1 if i % 8 else i + 7 for i in range(32)]
PADW = 144         # padded alpha tile width (junk cols >= 74 grow unbounded)
MULW = 72          # mult op width (cols 2:74), pads DVE pipe past RAW depth
ERW = 568          # ER8 flat width (8*TW data + 8 pad cols at 1.0)


def _build(Tmax, measures, G, NR):
    nc = bass.Bass()
    x_d = nc.declare_dram_parameter("x", [NB, TQ, TK], F32, isOutput=False)
    odd_d = nc.declare_dram_parameter("oddm", [128, NCH, 26], F32, isOutput=False)
    evpmfz_d = nc.declare_dram_parameter("evpmfz", [128, NCH, 26], F32, isOutput=False)
    fz_d = nc.declare_dram_parameter("fzp", [128, NCH, 26], F32, isOutput=False)
    liveb_d = nc.declare_dram_parameter("liveb", [128, G], F32, isOutput=False)
    biasb_d = nc.declare_dram_parameter("biasb", [128, G], F32, isOutput=False)
    initm_d = nc.declare_dram_parameter("initm", [128, TW], F32, isOutput=False)
    g1_d = nc.declare_dram_parameter("g1", [128, NB], F32, isOutput=False)
    g2_d = nc.declare_dram_parameter("g2", [NB, 128], F32, isOutput=False)
    alpha_d = nc.declare_dram_parameter("alpha", [128, TW], F32, isOutput=True)
    outsb_d = nc.declare_dram_parameter("outsb", [NB, max(NR, 1)], F32, isOutput=True)
    ss_d = nc.declare_dram_parameter("ssden", [128, G], F32, isOutput=True)
    emit_d = nc.dram_tensor("emitd", [G, 8, NB, NCH, TW], F32)

    stack = ExitStack()
    def sb(name, shape):
        return stack.enter_context(nc.sbuf_tensor(name, shape, F32))
    XR = [sb("xr%d" % i, [128, TK]) for i in range(4)]
    ER4 = [sb("er4_%d" % i, [128, NCH * 26]) for i in range(4)]
    EMR = [sb("emr%d" % i, [128, NCH, TW]) for i in range(3)]
    ER8 = [sb("er8_%d" % i, [128, ERW]) for i in range(3)]
    ODD = sb("oddsb", [128, NCH, 26])
    EVPMFZ = sb("evpmfzsb", [128, NCH, 26])
    FZ = sb("fzsb", [128, NCH, 26])
    LIVEB = sb("livebsb", [128, G])
    BIASB = sb("biasbsb", [128, G])
    INITM = sb("initmsb", [128, TW])
    G1S = sb("g1sb", [128, NB])
    G2S = sb("g2sb", [NB, 128])
    AB = [sb("ab%d" % i, [128, PADW]) for i in range(2)]
    MS = [sb("ms%d" % i, [128, 1]) for i in range(2)]
    INV = sb("invsb", [128, 1])
    OUTSB = sb("outsbsb", [NB, max(NR, 1)])
    SSD = sb("ssdsb", [128, G])
    PS1 = stack.enter_context(nc.psum_tensor("ps1t", [NB, 1], F32))
    PS2 = stack.enter_context(nc.psum_tensor("ps2t", [128, 1], F32))

    r_of = lambda g: min(8, Tmax - 8 * g)
    me_at = {t: m for m, t in enumerate(measures)}
    cp_at = {t + LAG // 2: m for m, t in enumerate(measures)}
    ap_at = {t + LAG: m for m, t in enumerate(measures)}

    xdma = [stack.enter_context(nc.semaphore("xdma%d" % i)) for i in range(4)]
    with (
        nc.Block() as block,
        nc.semaphore("cdma") as cdma,
        nc.semaphore("acts") as acts,
        nc.semaphore("evc") as evc,      # scalar even-pattern copy done
        nc.semaphore("pa") as pa,        # Pool finished A (ER4 consumed)
        nc.semaphore("pc") as pc,        # Pool finished tile build
        nc.semaphore("emst") as emst,    # emit store DMA complete
        nc.semaphore("elo") as elo,      # ER8 load complete
        nc.semaphore("cons") as cons,    # DVE consumed tile
        nc.semaphore("msem") as msem,    # measure accum ready
        nc.semaphore("g1sem") as g1sem,
        nc.semaphore("cpsem") as cpsem,
        nc.semaphore("g2sem") as g2sem,
        nc.semaphore("fin") as fin,
        nc.semaphore("ssfin") as ssfin,
        nc.semaphore("outd") as outd,
        nc.semaphore("dsync") as dsync,
    ):

        @block.sync
        def _(sync):
            for src, dst in [
                (odd_d, ODD), (evpmfz_d, EVPMFZ), (fz_d, FZ), (liveb_d, LIVEB),
                (biasb_d, BIASB), (initm_d, INITM), (g1_d, G1S), (g2_d, G2S),
            ]:
                sync.dma_start(out=dst[:], in_=src[:]).then_inc(cdma, 16)
            for g in range(G + 2):
                if g < G:
                    if g >= 4:
                        sync.wait_ge(acts, g - 3)
                    r = r_of(g)
                    sync.dma_start(
                        out=XR[g % 4][0:r * NB, :],
                        in_=x_d[:, 8 * g:8 * g + r, :].rearrange("b t k -> t b k"),
                    ).then_inc(xdma[g % 4], 16)
                if g >= 2:
                    # ER8 load for tile g-2 (after its store; ring depth 3)
                    gl = g - 2
                    sync.wait_ge(emst, 16 * (gl + 1))
                    if gl >= 3:
                        sync.wait_ge(cons, gl - 2)
                    sync.dma_start(
                        out=ER8[gl % 3][:, 0:8 * TW].rearrange(
                            "p (t w) -> p t w", t=8),
                        in_=emit_d[gl].rearrange("t b j w -> (b j) t w"),
                    ).then_inc(elo, 16)

        @block.scalar
        def _(scalar):
            scalar.wait_ge(cdma, 128)
            for g in range(G):
                scalar.wait_ge(xdma[g % 4], 16 * (g // 4 + 1))
                if g >= 4:
                    scalar.wait_ge(pa, g - 3)
                r = r_of(g)
                nc.scalar.activation(
                    out=ER4[g % 4][0:r * NB, 0:TK], in_=XR[g % 4][0:r * NB, :],
                    func=ACTF.Exp, bias=BIASB[0:r * NB, g:g + 1],
                    accum_out=SSD[0:r * NB, g:g + 1],
                ).then_inc(acts, 1)
                # even-state pattern, live-scaled (freeze handled by Pool +FZ)
                if g >= 3:
                    scalar.wait_ge(emst, 16 * (g - 2))
                nc.scalar.activation(
                    out=EMR[g % 3][0:r * NB, :, GW:TW:2], in_=EVPMFZ[0:r * NB],
                    func=ACTF.Copy, scale=LIVEB[0:r * NB, g:g + 1],
                ).then_inc(evc, 1)
                # issue store for tile g-1 (Pool done by then)
                if g >= 1:
                    scalar.wait_ge(pc, g)
                    rp = r_of(g - 1)
                    scalar.dma_start(
                        out=emit_d[g - 1][0:rp].rearrange("t b j w -> (t b) j w"),
                        in_=EMR[(g - 1) % 3][0:rp * NB],
                    ).then_inc(emst, 16)
            scalar.wait_ge(pc, G)
            rp = r_of(G - 1)
            scalar.dma_start(
                out=emit_d[G - 1][0:rp].rearrange("t b j w -> (t b) j w"),
                in_=EMR[(G - 1) % 3][0:rp * NB],
            ).then_inc(emst, 16)
            nc.scalar.activation(
                out=INV[:], in_=INV[:], func=ACTF.Copy).then_inc(ssfin, 1)

        @block.gpsimd
        def _(gpsimd):
            gpsimd.wait_ge(cdma, 128)
            for g in range(G):
                r = r_of(g)
                # B2: add freeze one-hot to the live-scaled even pattern
                gpsimd.wait_ge(evc, g + 1)
                nc.gpsimd.tensor_tensor(
                    out=EMR[g % 3][0:r * NB, :, GW:TW:2],
                    in0=EMR[g % 3][0:r * NB, :, GW:TW:2],
                    in1=FZ[0:r * NB], op=ALU.add)
                # A: odd-state emissions (dead rows killed via exp bias)
                gpsimd.wait_ge(acts, g + 1)
                nc.gpsimd.tensor_tensor(
                    out=EMR[g % 3][0:r * NB, :, GW + 1:TW:2],
                    in0=ER4[g % 4][0:r * NB].rearrange("p (j q) -> p j q", j=NCH),
                    in1=ODD[0:r * NB], op=ALU.mult).then_inc(pa, 1)
                # C: duplicate chunk tails into next chunk's ghost cols
                nc.gpsimd.tensor_copy(
                    out=EMR[g % 3][0:r * NB, 1:NCH, 0:GW],
                    in_=EMR[g % 3][0:r * NB, 0:NCH - 1, W:TW],
                ).then_inc(pc, 1)
            # final exports
            gpsimd.wait_ge(fin, 1)
            gpsimd.dma_start(out=alpha_d[:], in_=AB[(Tmax - 1) % 2][:, 0:TW]).then_inc(outd, 16)
            gpsimd.dma_start(out=outsb_d[:], in_=OUTSB[:]).then_inc(outd, 16)
            gpsimd.wait_ge(ssfin, 1)
            gpsimd.dma_start(out=ss_d[:], in_=SSD[:]).then_inc(outd, 16)

        @block.tensor
        def _(tensor):
            tensor.wait_ge(cdma, 128)
            for m in range(NR):
                tensor.wait_ge(msem, m + 1)
                nc.tensor.matmul(PS1[:], G1S[:], MS[m % 2][:],
                                 start=True, stop=True).then_inc(g1sem, 1)
                tensor.wait_ge(cpsem, m + 1)
                nc.tensor.matmul(PS2[:], G2S[:], OUTSB[:, m:m + 1],
                                 start=True, stop=True).then_inc(g2sem, 1)

        @block.vector
        def _(vector):
            vector.wait_ge(cdma, 128)
            for buf in AB:
                nc.vector.memset(buf[:], 0.0)
            nc.vector.memset(INV[:], 1.0)
            for er in ER8:
                nc.vector.memset(er[:, 8 * TW:ERW], 1.0)
            vector.wait_ge(elo, 16)
            # alpha_0 = E_0 * INITM  (states 0,1 of chunk 0)
            nc.vector.tensor_tensor(
                out=AB[0][:, 0:TW], in0=ER8[0][:, 0:TW], in1=INITM[:],
                op=ALU.mult)
            # Pad-mode main loop: wide ops provide in-pipe RAW spacing
            # (write-visibility depth ~66 DVE cycles). Only fix->mult and
            # the shuffle boundaries need explicit drains.
            for t in range(1, Tmax):
                g, tl = t // 8, t % 8
                CUR, NXT = AB[(t - 1) % 2], AB[t % 2]
                if (t - 1) % KSH == 0:
                    nc.vector.drain()
                    nc.vector.stream_shuffle(
                        out=CUR[:, 0:GW], in_=CUR[:, W:TW], mask=SHMASK)
                    nc.vector.drain()
                if tl == 0:
                    vector.wait_ge(elo, 16 * (g + 1))
                m_cp = cp_at.get(t)
                if m_cp is not None:
                    vector.wait_ge(g1sem, m_cp + 1)
                    nc.vector.tensor_copy(
                        out=OUTSB[:, m_cp:m_cp + 1], in_=PS1[:]).then_inc(cpsem, 1)
                m_ap = ap_at.get(t)
                if m_ap is not None:
                    vector.wait_ge(g2sem, m_ap + 1)
                    nc.vector.reciprocal(out=INV[:], in_=PS2[:])
                nc.vector.tensor_add(
                    NXT[:, 2:PADW], CUR[:, 2:PADW], CUR[:, 1:PADW - 1])
                nc.vector.tensor_add(
                    NXT[:, 3:PADW - 1:2], NXT[:, 3:PADW - 1:2],
                    CUR[:, 1:PADW - 3:2])
                m_me = me_at.get(t)
                if m_me is not None:
                    st = nc.vector.scalar_tensor_tensor(
                        out=NXT[:, 2:TW], in0=NXT[:, 2:TW],
                        scalar=(INV[:] if m_ap is not None else 1.0),
                        in1=ER8[g % 3][:, tl * TW + 2:tl * TW + TW],
                        op0=ALU.mult, op1=ALU.mult, accum_out=MS[m_me % 2][:])
                    nc.vector.drain()
                    st.then_inc(msem, 1)
                else:
                    st = nc.vector.scalar_tensor_tensor(
                        out=NXT[:, 2:2 + MULW], in0=NXT[:, 2:2 + MULW],
                        scalar=(INV[:] if m_ap is not None else 1.0),
                        in1=ER8[g % 3][:, tl * TW + 2:tl * TW + 2 + MULW],
                        op0=ALU.mult, op1=ALU.mult)
                if tl == 7 or t == Tmax - 1:
                    st.then_inc(cons, 1)
            nc.vector.memset(AB[(Tmax - 1) % 2][:, 0:1], 0.0).then_inc(fin, 1)

    stack.close()
    return nc


def _host_constants(in_lens_c, out_lens_c, Tmax, G):
    """Per-core constant tensors. Partition p = b*8 + j; stage-A rows (tl*16+b)."""
    b_row = np.arange(128) % NB             # stage-A row -> sample
    tl_row = np.arange(128) // NB           # stage-A row -> local t
    L = in_lens_c.astype(np.int64)          # [16]
    twoL = 2 * L
    # state grid per (j, q): odd states s = 52j + 2q + 1, even states s = 52j + 2q
    j_g, q_g = np.meshgrid(np.arange(NCH), np.arange(26), indexing="ij")
    s_odd = 52 * j_g + 2 * q_g + 1          # [8, 26]
    s_even = 52 * j_g + 2 * q_g
    odd = (s_odd[None] <= twoL[:, None, None]).astype(np.float32)       # [16,8,26]
    evp = EB * (s_even[None] <= twoL[:, None, None]).astype(np.float32)
    fz = (s_even[None] == twoL[:, None, None]).astype(np.float32)
    oddm = odd[b_row].astype(np.float32).copy()
    evpmfz = (evp - fz)[b_row].astype(np.float32).copy()
    fzp = fz[b_row].astype(np.float32).copy()
    # live[(tl*16+b), g] = t < ol_b with t = 8g + tl
    tt = 8 * np.arange(G)[None, :] + tl_row[:, None]                    # [128, G]
    liveb = (tt < out_lens_c[b_row][:, None]).astype(np.float32)
    biasb = ((liveb - 1.0) * 80.0).astype(np.float32)
    initm = np.zeros((128, TW), np.float32)
    p_j = np.arange(128) % NCH
    initm[(p_j == 0), GW] = 1.0
    initm[(p_j == 0), GW + 1] = 1.0
    p_b = np.arange(128) // NCH
    g1 = (p_b[:, None] == np.arange(NB)[None, :]).astype(np.float32)
    g2 = (np.arange(NB)[:, None] == p_b[None, :]).astype(np.float32)
    return dict(oddm=oddm, evpmfz=evpmfz, fzp=fzp, liveb=liveb, biasb=biasb,
                initm=initm, g1=g1, g2=g2)


def kernel(attn_logprob, in_lens, out_lens):
    x = np.ascontiguousarray(np.asarray(attn_logprob, np.float32)[:, 0])  # [128,900,200]
    il = np.asarray(in_lens).astype(np.int64)
    ol = np.asarray(out_lens).astype(np.int64)
    Bfull = x.shape[0]
    Tmax = int(ol.max())
    G = (Tmax + 7) // 8
    measures = [t for t in range(4, Tmax - 1 - LAG, RESC)]
    NR = len(measures)

    nc = _build(Tmax, measures, G, NR)

    in_maps = []
    for c in range(NCORES):
        sl = slice(c * NB, (c + 1) * NB)
        m = {"x": np.ascontiguousarray(x[sl])}
        m.update(_host_constants(il[sl], ol[sl], Tmax, G))
        in_maps.append(m)

    import os
    global LAST_RESULTS, LAST_EXEC_S
    LAST_EXEC_S = None
    LAST_RESULTS = run_bass_kernel_spmd(nc, in_maps, list(range(NCORES)))
    res = LAST_RESULTS.results
    if os.environ.get("BASS_PROFILE", "0") == "1":
        try:
            tdir = os.environ.get("BASS_TRACE_DIR") or None
            tr = run_bass_kernel_spmd(nc, in_maps, list(range(NCORES)),
                                      trace=True, tmpdir=tdir)
            if tr.exec_time_ns is not None:
                LAST_EXEC_S = tr.exec_time_ns * 1e-9
                LAST_RESULTS = tr
        except Exception as e:
            print("trace run failed:", e)
        if LAST_EXEC_S is None:
            import time
            ts = []
            for _ in range(3):
                t0 = time.time()
                run_bass_kernel_spmd(nc, in_maps, list(range(NCORES)))
                ts.append(time.time() - t0)
            LAST_EXEC_S = min(ts)

    # host reconstruction
    losses = []
    for c in range(NCORES):
        sl = slice(c * NB, (c + 1) * NB)
        alpha = np.asarray(res[c]["alpha"], np.float64)    # [128, TW]
        outsb = np.asarray(res[c]["outsb"], np.float64)    # [16, NR]
        ss = np.asarray(res[c]["ssden"], np.float64)       # [128, G]
        Lc = il[sl]
        olc = ol[sl]
        lb = np.zeros(NB)
        for b in range(NB):
            # denominators for frames t < ol_b
            ts_ = np.arange(olc[b])
            D = ss[(ts_ % 8) * NB + b, ts_ // 8] + EB
            lnD = np.sum(np.log(np.maximum(D, 1e-300)))
            lnM = np.sum(np.log(np.maximum(outsb[b, :NR], 1e-300)))
            s_hi = 2 * Lc[b]
            v = alpha[b * NCH + s_hi // W, GW + s_hi % W]
            if olc[b] == Tmax:
                s_lo = s_hi - 1
                v = v + alpha[b * NCH + s_lo // W, GW + s_lo % W]
            with np.errstate(divide="ignore", invalid="ignore"):
                ln_true = np.log(v) - lnD + lnM
            loss = -ln_true / Lc[b]
            if not np.isfinite(loss) or loss > 1e20:
                loss = 0.0
            lb[b] = loss
        losses.append(lb)
    return np.float32(np.mean(np.concatenate(losses)[:Bfull]))


# revision 32
# speedup vs baseline: 1.8354x; 1.0674x over previous
"""Bass/Trainium2 kernel for nn_AttentionCTCLoss (RAD-TTS attention CTC loss).

Pure data-parallel over 8 NeuronCores (16 samples each). Per core:
softmax over 201 classes kept UNNORMALIZED (raw exp; per-frame denominators
exported, folded out on host). The 900-step CTC alpha recursion runs in the
probability domain on a chunked layout: partitions p = b*8 + j, chunk j holds
states [52j, 52j+52) plus a 10-col ghost halo [52j-10, 52j) refreshed every 4
steps with a single DVE stream_shuffle (partition+1 copy within quadrants).
Per step the DVE does exactly 3 ops (add, strided skip-add, STT multiply by
emissions with rescale factor folded in). Per-sample rescale every 12 steps
via PE matmuls (measure accum -> G1 -> G2 -> reciprocal), applied with an
8-step lag. Final-state extraction uses an emission "freeze": for t >= ol_b
the emission row becomes one-hot 1.0 at state 2L_b, which makes
alpha[2L_b] = hi+lo and keeps it fixed, so no per-step snapshots are needed.
Host reconstructs loss_b = -(ln alpha[2L] - sum ln D_t + sum ln M_k) / L_b.
"""
import math
import numpy as np
from contextlib import ExitStack

import concourse.bass as bass
import concourse.mybir as mybir
from concourse.bass_utils import run_bass_kernel_spmd

LAST_RESULTS = None
LAST_EXEC_S = None

F32 = mybir.dt.float32
ALU = mybir.AluOpType
ACTF = mybir.ActivationFunctionType

NCORES = 8
NB = 16            # samples per core
TQ, TK = 900, 200
NCH = 8            # state chunks per sample
W = 52             # interior states per chunk (8*52 = 416 >= 401)
GW = 18            # ghost halo columns
TW = GW + W        # tile width 62
KSH = 8            # ghost refresh (stream_shuffle) period
RESC = 24          # rescale period
LAG = 8            # measure -> apply lag
EB = math.exp(-1.0)
SHMASK = [i - 1 if i % 8 else i + 7 for i in range(32)]
PADW = 136         # alpha tile width incl junk pad cols (>= 74 junk)
TAW = 104          # T-add op span end (covers real cols + pipe pad)
MULW = 72          # mult op width (cols 2:74), pads DVE pipe past RAW depth
ERW = 568          # ER8 flat width (8*TW data + 8 pad cols at 1.0)


def _build(Tmax, measures, G, NR):
    nc = bass.Bass()
    x_d = nc.declare_dram_parameter("x", [NB, TQ, TK], F32, isOutput=False)
    odd_d = nc.declare_dram_parameter("oddm", [128, NCH, 26], F32, isOutput=False)
    evpmfz_d = nc.declare_dram_parameter("evpmfz", [128, NCH, 26], F32, isOutput=False)
    fz_d = nc.declare_dram_parameter("fzp", [128, NCH, 26], F32, isOutput=False)
    liveb_d = nc.declare_dram_parameter("liveb", [128, G], F32, isOutput=False)
    biasb_d = nc.declare_dram_parameter("biasb", [128, G], F32, isOutput=False)
    initm_d = nc.declare_dram_parameter("initm", [128, TW], F32, isOutput=False)
    g1_d = nc.declare_dram_parameter("g1", [128, NB], F32, isOutput=False)
    g2_d = nc.declare_dram_parameter("g2", [NB, 128], F32, isOutput=False)
    alpha_d = nc.declare_dram_parameter("alpha", [128, TW], F32, isOutput=True)
    outsb_d = nc.declare_dram_parameter("outsb", [NB, max(NR, 1)], F32, isOutput=True)
    ss_d = nc.declare_dram_parameter("ssden", [128, G], F32, isOutput=True)
    emit_d = nc.dram_tensor("emitd", [G, 8, NB, NCH, TW], F32)

    stack = ExitStack()
    def sb(name, shape):
        return stack.enter_context(nc.sbuf_tensor(name, shape, F32))
    XR = [sb("xr%d" % i, [128, TK]) for i in range(4)]
    ER4 = [sb("er4_%d" % i, [128, NCH * 26]) for i in range(4)]
    EMR = [sb("emr%d" % i, [128, NCH, TW]) for i in range(3)]
    ER8 = [sb("er8_%d" % i, [128, ERW]) for i in range(3)]
    ODD = sb("oddsb", [128, NCH, 26])
    EVPMFZ = sb("evpmfzsb", [128, NCH, 26])
    FZ = sb("fzsb", [128, NCH, 26])
    LIVEB = sb("livebsb", [128, G])
    BIASB = sb("biasbsb", [128, G])
    INITM = sb("initmsb", [128, TW])
    G1S = sb("g1sb", [128, NB])
    G2S = sb("g2sb", [NB, 128])
    AB = [sb("ab%d" % i, [128, PADW]) for i in range(2)]
    MS = [sb("ms%d" % i, [128, 1]) for i in range(2)]
    INV = sb("invsb", [128, 1])
    OUTSB = sb("outsbsb", [NB, max(NR, 1)])
    SSD = sb("ssdsb", [128, G])
    PS1 = stack.enter_context(nc.psum_tensor("ps1t", [NB, 1], F32))
    PS2 = stack.enter_context(nc.psum_tensor("ps2t", [128, 1], F32))

    r_of = lambda g: min(8, Tmax - 8 * g)
    me_at = {t: m for m, t in enumerate(measures)}
    cp_at = {t + LAG // 2: m for m, t in enumerate(measures)}
    ap_at = {t + LAG: m for m, t in enumerate(measures)}

    xdma = [stack.enter_context(nc.semaphore("xdma%d" % i)) for i in range(4)]
    with (
        nc.Block() as block,
        nc.semaphore("cdma") as cdma,
        nc.semaphore("acts") as acts,
        nc.semaphore("evc") as evc,      # scalar even-pattern copy done
        nc.semaphore("pa") as pa,        # Pool finished A (ER4 consumed)
        nc.semaphore("pc") as pc,        # Pool finished tile build
        nc.semaphore("emst") as emst,    # emit store DMA complete
        nc.semaphore("elo") as elo,      # ER8 load complete
        nc.semaphore("cons") as cons,    # DVE consumed tile
        nc.semaphore("msem") as msem,    # measure accum ready
        nc.semaphore("g1sem") as g1sem,
        nc.semaphore("cpsem") as cpsem,
        nc.semaphore("g2sem") as g2sem,
        nc.semaphore("fin") as fin,
        nc.semaphore("ssfin") as ssfin,
        nc.semaphore("outd") as outd,
        nc.semaphore("dsync") as dsync,
    ):

        @block.sync
        def _(sync):
            for src, dst in [
                (odd_d, ODD), (evpmfz_d, EVPMFZ), (fz_d, FZ), (liveb_d, LIVEB),
                (biasb_d, BIASB), (initm_d, INITM), (g1_d, G1S), (g2_d, G2S),
            ]:
                sync.dma_start(out=dst[:], in_=src[:]).then_inc(cdma, 16)
            for g in range(G + 2):
                if g < G:
                    if g >= 4:
                        sync.wait_ge(acts, g - 3)
                    r = r_of(g)
                    sync.dma_start(
                        out=XR[g % 4][0:r * NB, :],
                        in_=x_d[:, 8 * g:8 * g + r, :].rearrange("b t k -> t b k"),
                    ).then_inc(xdma[g % 4], 16)
                if g >= 2:
                    # ER8 load for tile g-2 (after its store; ring depth 3)
                    gl = g - 2
                    sync.wait_ge(emst, 16 * (gl + 1))
                    if gl >= 3:
                        sync.wait_ge(cons, gl - 2)
                    sync.dma_start(
                        out=ER8[gl % 3][:, 0:8 * TW].rearrange(
                            "p (t w) -> p t w", t=8),
                        in_=emit_d[gl].rearrange("t b j w -> (b j) t w"),
                    ).then_inc(elo, 16)

        @block.scalar
        def _(scalar):
            scalar.wait_ge(cdma, 128)
            for g in range(G):
                scalar.wait_ge(xdma[g % 4], 16 * (g // 4 + 1))
                if g >= 4:
                    scalar.wait_ge(pa, g - 3)
                r = r_of(g)
                nc.scalar.activation(
                    out=ER4[g % 4][0:r * NB, 0:TK], in_=XR[g % 4][0:r * NB, :],
                    func=ACTF.Exp, bias=BIASB[0:r * NB, g:g + 1],
                    accum_out=SSD[0:r * NB, g:g + 1],
                ).then_inc(acts, 1)
                # even-state pattern, live-scaled (freeze handled by Pool +FZ)
                if g >= 3:
                    scalar.wait_ge(emst, 16 * (g - 2))
                nc.scalar.activation(
                    out=EMR[g % 3][0:r * NB, :, GW:TW:2], in_=EVPMFZ[0:r * NB],
                    func=ACTF.Copy, scale=LIVEB[0:r * NB, g:g + 1],
                ).then_inc(evc, 1)
                # issue store for tile g-1 (Pool done by then)
                if g >= 1:
                    scalar.wait_ge(pc, g)
                    rp = r_of(g - 1)
                    scalar.dma_start(
                        out=emit_d[g - 1][0:rp].rearrange("t b j w -> (t b) j w"),
                        in_=EMR[(g - 1) % 3][0:rp * NB],
                    ).then_inc(emst, 16)
            scalar.wait_ge(pc, G)
            rp = r_of(G - 1)
            scalar.dma_start(
                out=emit_d[G - 1][0:rp].rearrange("t b j w -> (t b) j w"),
                in_=EMR[(G - 1) % 3][0:rp * NB],
            ).then_inc(emst, 16)
            nc.scalar.activation(
                out=INV[:], in_=INV[:], func=ACTF.Copy).then_inc(ssfin, 1)

        @block.gpsimd
        def _(gpsimd):
            gpsimd.wait_ge(cdma, 128)
            for g in range(G):
                r = r_of(g)
                # B2: add freeze one-hot to the live-scaled even pattern
                gpsimd.wait_ge(evc, g + 1)
                nc.gpsimd.tensor_tensor(
                    out=EMR[g % 3][0:r * NB, :, GW:TW:2],
                    in0=EMR[g % 3][0:r * NB, :, GW:TW:2],
                    in1=FZ[0:r * NB], op=ALU.add)
                # A: odd-state emissions (dead rows killed via exp bias)
                gpsimd.wait_ge(acts, g + 1)
                nc.gpsimd.tensor_tensor(
                    out=EMR[g % 3][0:r * NB, :, GW + 1:TW:2],
                    in0=ER4[g % 4][0:r * NB].rearrange("p (j q) -> p j q", j=NCH),
                    in1=ODD[0:r * NB], op=ALU.mult).then_inc(pa, 1)
                # C: duplicate chunk tails into next chunk's ghost cols
                nc.gpsimd.tensor_copy(
                    out=EMR[g % 3][0:r * NB, 1:NCH, 0:GW],
                    in_=EMR[g % 3][0:r * NB, 0:NCH - 1, W:TW],
                ).then_inc(pc, 1)
            # final exports
            gpsimd.wait_ge(fin, 1)
            gpsimd.dma_start(out=alpha_d[:], in_=AB[(Tmax - 1) % 2][:, 0:TW]).then_inc(outd, 16)
            gpsimd.dma_start(out=outsb_d[:], in_=OUTSB[:]).then_inc(outd, 16)
            gpsimd.wait_ge(ssfin, 1)
            gpsimd.dma_start(out=ss_d[:], in_=SSD[:]).then_inc(outd, 16)

        @block.tensor
        def _(tensor):
            tensor.wait_ge(cdma, 128)
            for m in range(NR):
                tensor.wait_ge(msem, m + 1)
                nc.tensor.matmul(PS1[:], G1S[:], MS[m % 2][:],
                                 start=True, stop=True).then_inc(g1sem, 1)
                tensor.wait_ge(cpsem, m + 1)
                nc.tensor.matmul(PS2[:], G2S[:], OUTSB[:, m:m + 1],
                                 start=True, stop=True).then_inc(g2sem, 1)

        @block.vector
        def _(vector):
            vector.wait_ge(cdma, 128)
            for buf in AB:
                nc.vector.memset(buf[:], 0.0)
            nc.vector.memset(INV[:], 1.0)
            for er in ER8:
                nc.vector.memset(er[:, 8 * TW:ERW], 1.0)
            vector.wait_ge(elo, 16)
            # alpha_0 = E_0 * INITM  (states 0,1 of chunk 0)
            nc.vector.tensor_tensor(
                out=AB[0][:, 0:TW], in0=ER8[0][:, 0:TW], in1=INITM[:],
                op=ALU.mult)
            # Pad-mode main loop: wide ops provide in-pipe RAW spacing
            # (write-visibility depth ~66 DVE cycles). Only fix->mult and
            # the shuffle boundaries need explicit drains.
            for t in range(1, Tmax):
                g, tl = t // 8, t % 8
                CUR, NXT = AB[(t - 1) % 2], AB[t % 2]
                if (t - 1) % KSH == 0:
                    nc.vector.drain()
                    nc.vector.stream_shuffle(
                        out=CUR[:, 0:GW], in_=CUR[:, W:TW], mask=SHMASK)
                    nc.vector.drain()
                if tl == 0:
                    vector.wait_ge(elo, 16 * (g + 1))
                m_cp = cp_at.get(t)
                if m_cp is not None:
                    vector.wait_ge(g1sem, m_cp + 1)
                    nc.vector.tensor_copy(
                        out=OUTSB[:, m_cp:m_cp + 1], in_=PS1[:]).then_inc(cpsem, 1)
                m_ap = ap_at.get(t)
                if m_ap is not None:
                    vector.wait_ge(g2sem, m_ap + 1)
                    nc.vector.reciprocal(out=INV[:], in_=PS2[:])
                nc.vector.tensor_add(
                    NXT[:, 2:TAW], CUR[:, 2:TAW], CUR[:, 1:TAW - 1])
                nc.vector.tensor_add(
                    NXT[:, 3:PADW - 1:2], NXT[:, 3:PADW - 1:2],
                    CUR[:, 1:PADW - 3:2])
                m_me = me_at.get(t)
                kw = {}
                if m_me is not None:
                    kw["accum_out"] = MS[m_me % 2][:]
                st = nc.vector.scalar_tensor_tensor(
                    out=NXT[:, 2:2 + MULW], in0=NXT[:, 2:2 + MULW],
                    scalar=(INV[:] if m_ap is not None else 1.0),
                    in1=ER8[g % 3][:, tl * TW + 2:tl * TW + 2 + MULW],
                    op0=ALU.mult, op1=ALU.mult, **kw)
                if m_me is not None:
                    st.then_inc(msem, 1)
                if tl == 7 or t == Tmax - 1:
                    st.then_inc(cons, 1)
            nc.vector.memset(AB[(Tmax - 1) % 2][:, 0:1], 0.0).then_inc(fin, 1)

    stack.close()
    return nc


def _host_constants(in_lens_c, out_lens_c, Tmax, G):
    """Per-core constant tensors. Partition p = b*8 + j; stage-A rows (tl*16+b)."""
    b_row = np.arange(128) % NB             # stage-A row -> sample
    tl_row = np.arange(128) // NB           # stage-A row -> local t
    L = in_lens_c.astype(np.int64)          # [16]
    twoL = 2 * L
    # state grid per (j, q): odd states s = 52j + 2q + 1, even states s = 52j + 2q
    j_g, q_g = np.meshgrid(np.arange(NCH), np.arange(26), indexing="ij")
    s_odd = 52 * j_g + 2 * q_g + 1          # [8, 26]
    s_even = 52 * j_g + 2 * q_g
    odd = (s_odd[None] <= twoL[:, None, None]).astype(np.float32)       # [16,8,26]
    evp = EB * (s_even[None] <= twoL[:, None, None]).astype(np.float32)
    fz = (s_even[None] == twoL[:, None, None]).astype(np.float32)
    oddm = odd[b_row].astype(np.float32).copy()
    evpmfz = (evp - fz)[b_row].astype(np.float32).copy()
    fzp = fz[b_row].astype(np.float32).copy()
    # live[(tl*16+b), g] = t < ol_b with t = 8g + tl
    tt = 8 * np.arange(G)[None, :] + tl_row[:, None]                    # [128, G]
    liveb = (tt < out_lens_c[b_row][:, None]).astype(np.float32)
    biasb = ((liveb - 1.0) * 80.0).astype(np.float32)
    initm = np.zeros((128, TW), np.float32)
    p_j = np.arange(128) % NCH
    initm[(p_j == 0), GW] = 1.0
    initm[(p_j == 0), GW + 1] = 1.0
    p_b = np.arange(128) // NCH
    g1 = (p_b[:, None] == np.arange(NB)[None, :]).astype(np.float32)
    g2 = (np.arange(NB)[:, None] == p_b[None, :]).astype(np.float32)
    return dict(oddm=oddm, evpmfz=evpmfz, fzp=fzp, liveb=liveb, biasb=biasb,
                initm=initm, g1=g1, g2=g2)


def kernel(attn_logprob, in_lens, out_lens):
    x = np.ascontiguousarray(np.asarray(attn_logprob, np.float32)[:, 0])  # [128,900,200]
    il = np.asarray(in_lens).astype(np.int64)
    ol = np.asarray(out_lens).astype(np.int64)
    Bfull = x.shape[0]
    Tmax = int(ol.max())
    G = (Tmax + 7) // 8
    measures = [t for t in range(4, Tmax - 1 - LAG, RESC)]
    NR = len(measures)

    nc = _build(Tmax, measures, G, NR)

    in_maps = []
    for c in range(NCORES):
        sl = slice(c * NB, (c + 1) * NB)
        m = {"x": np.ascontiguousarray(x[sl])}
        m.update(_host_constants(il[sl], ol[sl], Tmax, G))
        in_maps.append(m)

    import os
    global LAST_RESULTS, LAST_EXEC_S
    LAST_EXEC_S = None
    LAST_RESULTS = run_bass_kernel_spmd(nc, in_maps, list(range(NCORES)))
    res = LAST_RESULTS.results
    if os.environ.get("BASS_PROFILE", "0") == "1":
        try:
            tdir = os.environ.get("BASS_TRACE_DIR") or None
            tr = run_bass_kernel_spmd(nc, in_maps, list(range(NCORES)),
                                      trace=True, tmpdir=tdir)
            if tr.exec_time_ns is not None:
                LAST_EXEC_S = tr.exec_time_ns * 1e-9
                LAST_RESULTS = tr
        except Exception as e:
            print("trace run failed:", e)
        if LAST_EXEC_S is None:
            import time
            ts = []
            for _ in range(3):
                t0 = time.time()
                run_bass_kernel_spmd(nc, in_maps, list(range(NCORES)))
                ts.append(time.time() - t0)
            LAST_EXEC_S = min(ts)

    # host reconstruction
    losses = []
    for c in range(NCORES):
        sl = slice(c * NB, (c + 1) * NB)
        alpha = np.asarray(res[c]["alpha"], np.float64)    # [128, TW]
        outsb = np.asarray(res[c]["outsb"], np.float64)    # [16, NR]
        ss = np.asarray(res[c]["ssden"], np.float64)       # [128, G]
        Lc = il[sl]
        olc = ol[sl]
        lb = np.zeros(NB)
        for b in range(NB):
            # denominators for frames t < ol_b
            ts_ = np.arange(olc[b])
            D = ss[(ts_ % 8) * NB + b, ts_ // 8] + EB
            lnD = np.sum(np.log(np.maximum(D, 1e-300)))
            lnM = np.sum(np.log(np.maximum(outsb[b, :NR], 1e-300)))
            s_hi = 2 * Lc[b]
            v = alpha[b * NCH + s_hi // W, GW + s_hi % W]
            if olc[b] == Tmax:
                s_lo = s_hi - 1
                v = v + alpha[b * NCH + s_lo // W, GW + s_lo % W]
            with np.errstate(divide="ignore", invalid="ignore"):
                ln_true = np.log(v) - lnD + lnM
            loss = -ln_true / Lc[b]
            if not np.isfinite(loss) or loss > 1e20:
                loss = 0.0
            lb[b] = loss
        losses.append(lb)
    return np.float32(np.mean(np.concatenate(losses)[:Bfull]))


# revision 36
# speedup vs baseline: 1.9090x; 1.0400x over previous
"""Bass/Trainium2 kernel for nn_AttentionCTCLoss (RAD-TTS attention CTC loss).

Pure data-parallel over 8 NeuronCores (16 samples each). Per core:
softmax over 201 classes kept UNNORMALIZED (raw exp; per-frame denominators
exported, folded out on host). The 900-step CTC alpha recursion runs in the
probability domain on a chunked layout: partitions p = b*8 + j, chunk j holds
states [52j, 52j+52) plus a 10-col ghost halo [52j-10, 52j) refreshed every 4
steps with a single DVE stream_shuffle (partition+1 copy within quadrants).
Per step the DVE does exactly 3 ops (add, strided skip-add, STT multiply by
emissions with rescale factor folded in). Per-sample rescale every 12 steps
via PE matmuls (measure accum -> G1 -> G2 -> reciprocal), applied with an
8-step lag. Final-state extraction uses an emission "freeze": for t >= ol_b
the emission row becomes one-hot 1.0 at state 2L_b, which makes
alpha[2L_b] = hi+lo and keeps it fixed, so no per-step snapshots are needed.
Host reconstructs loss_b = -(ln alpha[2L] - sum ln D_t + sum ln M_k) / L_b.
"""
import math
import numpy as np
import ml_dtypes
from contextlib import ExitStack

import concourse.bass as bass
import concourse.mybir as mybir
from concourse.bass_utils import run_bass_kernel_spmd

LAST_RESULTS = None
LAST_EXEC_S = None

F32 = mybir.dt.float32
BF16 = mybir.dt.bfloat16
ALU = mybir.AluOpType
ACTF = mybir.ActivationFunctionType

NCORES = 8
NB = 16            # samples per core
TQ, TK = 900, 200
NCH = 8            # state chunks per sample
W = 52             # interior states per chunk (8*52 = 416 >= 401)
GW = 18            # ghost halo columns
TW = GW + W        # tile width 62
KSH = 8            # ghost refresh (stream_shuffle) period
RESC = 24          # rescale period
LAG = 8            # measure -> apply lag
EB = math.exp(-1.0)
SHMASK = [i - 1 if i % 8 else i + 7 for i in range(32)]
PADW = 136         # alpha tile width incl junk pad cols (>= 74 junk)
TAW = 104          # T-add op span end (covers real cols + pipe pad)
MULW = 72          # mult op width (cols 2:74), pads DVE pipe past RAW depth
ERW = 568          # ER8 flat width (8*TW data + 8 pad cols at 1.0)


def _build(Tmax, measures, G, NR):
    nc = bass.Bass()
    x_d = nc.declare_dram_parameter("x", [NB, TQ, TK], BF16, isOutput=False)
    odd_d = nc.declare_dram_parameter("oddm", [128, NCH, 26], BF16, isOutput=False)
    evpmfz_d = nc.declare_dram_parameter("evpmfz", [128, NCH, 26], BF16, isOutput=False)
    fz_d = nc.declare_dram_parameter("fzp", [128, NCH, 26], BF16, isOutput=False)
    liveb_d = nc.declare_dram_parameter("liveb", [128, G], F32, isOutput=False)
    biasb_d = nc.declare_dram_parameter("biasb", [128, G], F32, isOutput=False)
    initm_d = nc.declare_dram_parameter("initm", [128, TW], F32, isOutput=False)
    g1_d = nc.declare_dram_parameter("g1", [128, NB], F32, isOutput=False)
    g2_d = nc.declare_dram_parameter("g2", [NB, 128], F32, isOutput=False)
    alpha_d = nc.declare_dram_parameter("alpha", [128, TW], F32, isOutput=True)
    outsb_d = nc.declare_dram_parameter("outsb", [NB, max(NR, 1)], F32, isOutput=True)
    ss_d = nc.declare_dram_parameter("ssden", [128, G], F32, isOutput=True)
    emit_d = nc.dram_tensor("emitd", [G, 8, NB, NCH, TW], BF16)

    stack = ExitStack()
    def sb(name, shape, dt=F32):
        return stack.enter_context(nc.sbuf_tensor(name, shape, dt))
    XR = [sb("xr%d" % i, [128, TK], BF16) for i in range(4)]
    ER4 = [sb("er4_%d" % i, [128, NCH * 26], BF16) for i in range(4)]
    EMR = [sb("emr%d" % i, [128, NCH, TW], BF16) for i in range(3)]
    ER8 = [sb("er8_%d" % i, [128, ERW], BF16) for i in range(3)]
    ER8F = [sb("er8f_%d" % i, [128, ERW]) for i in range(3)]
    ODD = sb("oddsb", [128, NCH, 26], BF16)
    EVPMFZ = sb("evpmfzsb", [128, NCH, 26], BF16)
    FZ = sb("fzsb", [128, NCH, 26], BF16)
    LIVEB = sb("livebsb", [128, G])
    BIASB = sb("biasbsb", [128, G])
    INITM = sb("initmsb", [128, TW])
    G1S = sb("g1sb", [128, NB])
    G2S = sb("g2sb", [NB, 128])
    AB = [sb("ab%d" % i, [128, PADW]) for i in range(2)]
    MS = [sb("ms%d" % i, [128, 1]) for i in range(2)]
    INV = sb("invsb", [128, 1])
    OUTSB = sb("outsbsb", [NB, max(NR, 1)])
    SSD = sb("ssdsb", [128, G])
    PS1 = stack.enter_context(nc.psum_tensor("ps1t", [NB, 1], F32))
    PS2 = stack.enter_context(nc.psum_tensor("ps2t", [128, 1], F32))

    r_of = lambda g: min(8, Tmax - 8 * g)
    me_at = {t: m for m, t in enumerate(measures)}
    cp_at = {t + LAG // 2: m for m, t in enumerate(measures)}
    ap_at = {t + LAG: m for m, t in enumerate(measures)}

    xdma = [stack.enter_context(nc.semaphore("xdma%d" % i)) for i in range(4)]
    with (
        nc.Block() as block,
        nc.semaphore("cdma") as cdma,
        nc.semaphore("acts") as acts,
        nc.semaphore("evc") as evc,      # scalar even-pattern copy done
        nc.semaphore("pa") as pa,        # Pool finished A (ER4 consumed)
        nc.semaphore("pc") as pc,        # Pool finished tile build
        nc.semaphore("emst") as emst,    # emit store DMA complete
        nc.semaphore("elo") as elo,      # ER8 load complete
        nc.semaphore("elo2") as elo2,    # ER8F f32 convert done
        nc.semaphore("cons") as cons,    # DVE consumed tile
        nc.semaphore("msem") as msem,    # measure accum ready
        nc.semaphore("g1sem") as g1sem,
        nc.semaphore("cpsem") as cpsem,
        nc.semaphore("g2sem") as g2sem,
        nc.semaphore("fin") as fin,
        nc.semaphore("ssfin") as ssfin,
        nc.semaphore("outd") as outd,
        nc.semaphore("dsync") as dsync,
    ):

        @block.sync
        def _(sync):
            for src, dst in [
                (odd_d, ODD), (evpmfz_d, EVPMFZ), (fz_d, FZ), (liveb_d, LIVEB),
                (biasb_d, BIASB), (initm_d, INITM), (g1_d, G1S), (g2_d, G2S),
            ]:
                sync.dma_start(out=dst[:], in_=src[:]).then_inc(cdma, 16)
            for g in range(G + 2):
                if g < G:
                    if g >= 4:
                        sync.wait_ge(acts, g - 3)
                    r = r_of(g)
                    sync.dma_start(
                        out=XR[g % 4][0:r * NB, :],
                        in_=x_d[:, 8 * g:8 * g + r, :].rearrange("b t k -> t b k"),
                    ).then_inc(xdma[g % 4], 16)
                if g >= 2:
                    # ER8 load for tile g-2 (after its store; ring depth 3)
                    gl = g - 2
                    sync.wait_ge(emst, 16 * (gl + 1))
                    if gl >= 3:
                        sync.wait_ge(cons, gl - 2)
                    sync.dma_start(
                        out=ER8[gl % 3][:, 0:8 * TW].rearrange(
                            "p (t w) -> p t w", t=8),
                        in_=emit_d[gl].rearrange("t b j w -> (b j) t w"),
                    ).then_inc(elo, 16)

        @block.scalar
        def _(scalar):
            scalar.wait_ge(cdma, 128)
            for g in range(G):
                scalar.wait_ge(xdma[g % 4], 16 * (g // 4 + 1))
                if g >= 4:
                    scalar.wait_ge(pa, g - 3)
                r = r_of(g)
                nc.scalar.activation(
                    out=ER4[g % 4][0:r * NB, 0:TK], in_=XR[g % 4][0:r * NB, :],
                    func=ACTF.Exp, bias=BIASB[0:r * NB, g:g + 1],
                    accum_out=SSD[0:r * NB, g:g + 1],
                ).then_inc(acts, 1)
                # even-state pattern, live-scaled (freeze handled by Pool +FZ)
                if g >= 3:
                    scalar.wait_ge(emst, 16 * (g - 2))
                nc.scalar.activation(
                    out=EMR[g % 3][0:r * NB, :, GW:TW:2], in_=EVPMFZ[0:r * NB],
                    func=ACTF.Copy, scale=LIVEB[0:r * NB, g:g + 1],
                ).then_inc(evc, 1)
                # issue store for tile g-1 (Pool done by then)
                if g >= 1:
                    scalar.wait_ge(pc, g)
                    rp = r_of(g - 1)
                    scalar.dma_start(
                        out=emit_d[g - 1][0:rp].rearrange("t b j w -> (t b) j w"),
                        in_=EMR[(g - 1) % 3][0:rp * NB],
                    ).then_inc(emst, 16)
                # upconvert loaded tile g-2 to f32 for the DVE
                if g >= 2:
                    gc = g - 2
                    scalar.wait_ge(elo, 16 * (gc + 1))
                    if gc >= 3:
                        scalar.wait_ge(cons, gc - 2)
                    nc.scalar.activation(
                        out=ER8F[gc % 3][:, 0:8 * TW],
                        in_=ER8[gc % 3][:, 0:8 * TW],
                        func=ACTF.Copy).then_inc(elo2, 1)
            scalar.wait_ge(pc, G)
            rp = r_of(G - 1)
            scalar.dma_start(
                out=emit_d[G - 1][0:rp].rearrange("t b j w -> (t b) j w"),
                in_=EMR[(G - 1) % 3][0:rp * NB],
            ).then_inc(emst, 16)
            for gc in (G - 2, G - 1):
                if gc >= 0:
                    scalar.wait_ge(elo, 16 * (gc + 1))
                    if gc >= 3:
                        scalar.wait_ge(cons, gc - 2)
                    nc.scalar.activation(
                        out=ER8F[gc % 3][:, 0:8 * TW],
                        in_=ER8[gc % 3][:, 0:8 * TW],
                        func=ACTF.Copy).then_inc(elo2, 1)
            nc.scalar.activation(
                out=INV[:], in_=INV[:], func=ACTF.Copy).then_inc(ssfin, 1)

        @block.gpsimd
        def _(gpsimd):
            gpsimd.wait_ge(cdma, 128)
            for g in range(G):
                r = r_of(g)
                # B2: add freeze one-hot to the live-scaled even pattern
                gpsimd.wait_ge(evc, g + 1)
                nc.gpsimd.tensor_tensor(
                    out=EMR[g % 3][0:r * NB, :, GW:TW:2],
                    in0=EMR[g % 3][0:r * NB, :, GW:TW:2],
                    in1=FZ[0:r * NB], op=ALU.add)
                # A: odd-state emissions (dead rows killed via exp bias)
                gpsimd.wait_ge(acts, g + 1)
                nc.gpsimd.tensor_tensor(
                    out=EMR[g % 3][0:r * NB, :, GW + 1:TW:2],
                    in0=ER4[g % 4][0:r * NB].rearrange("p (j q) -> p j q", j=NCH),
                    in1=ODD[0:r * NB], op=ALU.mult).then_inc(pa, 1)
                # C: duplicate chunk tails into next chunk's ghost cols
                nc.gpsimd.tensor_copy(
                    out=EMR[g % 3][0:r * NB, 1:NCH, 0:GW],
                    in_=EMR[g % 3][0:r * NB, 0:NCH - 1, W:TW],
                ).then_inc(pc, 1)
            # final exports
            gpsimd.wait_ge(fin, 1)
            gpsimd.dma_start(out=alpha_d[:], in_=AB[(Tmax - 1) % 2][:, 0:TW]).then_inc(outd, 16)
            gpsimd.dma_start(out=outsb_d[:], in_=OUTSB[:]).then_inc(outd, 16)
            gpsimd.wait_ge(ssfin, 1)
            gpsimd.dma_start(out=ss_d[:], in_=SSD[:]).then_inc(outd, 16)

        @block.tensor
        def _(tensor):
            tensor.wait_ge(cdma, 128)
            for m in range(NR):
                tensor.wait_ge(msem, m + 1)
                nc.tensor.matmul(PS1[:], G1S[:], MS[m % 2][:],
                                 start=True, stop=True).then_inc(g1sem, 1)
                tensor.wait_ge(cpsem, m + 1)
                nc.tensor.matmul(PS2[:], G2S[:], OUTSB[:, m:m + 1],
                                 start=True, stop=True).then_inc(g2sem, 1)

        @block.vector
        def _(vector):
            vector.wait_ge(cdma, 128)
            for buf in AB:
                nc.vector.memset(buf[:], 0.0)
            nc.vector.memset(INV[:], 1.0)
            for er in ER8F:
                nc.vector.memset(er[:, 8 * TW:ERW], 1.0)
            vector.wait_ge(elo2, 1)
            # alpha_0 = E_0 * INITM  (states 0,1 of chunk 0)
            nc.vector.tensor_tensor(
                out=AB[0][:, 0:TW], in0=ER8F[0][:, 0:TW], in1=INITM[:],
                op=ALU.mult)
            # Pad-mode main loop: wide ops provide in-pipe RAW spacing
            # (write-visibility depth ~66 DVE cycles). Only fix->mult and
            # the shuffle boundaries need explicit drains.
            for t in range(1, Tmax):
                g, tl = t // 8, t % 8
                CUR, NXT = AB[(t - 1) % 2], AB[t % 2]
                if (t - 1) % KSH == 0:
                    nc.vector.drain()
                    nc.vector.stream_shuffle(
                        out=CUR[:, 0:GW], in_=CUR[:, W:TW], mask=SHMASK)
                    nc.vector.drain()
                if tl == 0:
                    vector.wait_ge(elo2, g + 1)
                m_cp = cp_at.get(t)
                if m_cp is not None:
                    vector.wait_ge(g1sem, m_cp + 1)
                    nc.vector.tensor_copy(
                        out=OUTSB[:, m_cp:m_cp + 1], in_=PS1[:]).then_inc(cpsem, 1)
                m_ap = ap_at.get(t)
                if m_ap is not None:
                    vector.wait_ge(g2sem, m_ap + 1)
                    nc.vector.reciprocal(out=INV[:], in_=PS2[:])
                nc.vector.tensor_add(
                    NXT[:, 2:TAW], CUR[:, 2:TAW], CUR[:, 1:TAW - 1])
                nc.vector.tensor_add(
                    NXT[:, 3:PADW - 1:2], NXT[:, 3:PADW - 1:2],
                    CUR[:, 1:PADW - 3:2])
                m_me = me_at.get(t)
                kw = {}
                if m_me is not None:
                    kw["accum_out"] = MS[m_me % 2][:]
                st = nc.vector.scalar_tensor_tensor(
                    out=NXT[:, 2:2 + MULW], in0=NXT[:, 2:2 + MULW],
                    scalar=(INV[:] if m_ap is not None else 1.0),
                    in1=ER8F[g % 3][:, tl * TW + 2:tl * TW + 2 + MULW],
                    op0=ALU.mult, op1=ALU.mult, **kw)
                if m_me is not None:
                    st.then_inc(msem, 1)
                if tl == 7 or t == Tmax - 1:
                    st.then_inc(cons, 1)
            nc.vector.memset(AB[(Tmax - 1) % 2][:, 0:1], 0.0).then_inc(fin, 1)

    stack.close()
    return nc


def _host_constants(in_lens_c, out_lens_c, Tmax, G):
    """Per-core constant tensors. Partition p = b*8 + j; stage-A rows (tl*16+b)."""
    b_row = np.arange(128) % NB             # stage-A row -> sample
    tl_row = np.arange(128) // NB           # stage-A row -> local t
    L = in_lens_c.astype(np.int64)          # [16]
    twoL = 2 * L
    # state grid per (j, q): odd states s = 52j + 2q + 1, even states s = 52j + 2q
    j_g, q_g = np.meshgrid(np.arange(NCH), np.arange(26), indexing="ij")
    s_odd = 52 * j_g + 2 * q_g + 1          # [8, 26]
    s_even = 52 * j_g + 2 * q_g
    odd = (s_odd[None] <= twoL[:, None, None]).astype(np.float32)       # [16,8,26]
    evp = EB * (s_even[None] <= twoL[:, None, None]).astype(np.float32)
    fz = (s_even[None] == twoL[:, None, None]).astype(np.float32)
    oddm = odd[b_row].astype(ml_dtypes.bfloat16).copy()
    evpmfz = (evp - fz)[b_row].astype(ml_dtypes.bfloat16).copy()
    fzp = fz[b_row].astype(ml_dtypes.bfloat16).copy()
    # live[(tl*16+b), g] = t < ol_b with t = 8g + tl
    tt = 8 * np.arange(G)[None, :] + tl_row[:, None]                    # [128, G]
    liveb = (tt < out_lens_c[b_row][:, None]).astype(np.float32)
    biasb = ((liveb - 1.0) * 80.0).astype(np.float32)
    initm = np.zeros((128, TW), np.float32)
    p_j = np.arange(128) % NCH
    initm[(p_j == 0), GW] = 1.0
    initm[(p_j == 0), GW + 1] = 1.0
    p_b = np.arange(128) // NCH
    g1 = (p_b[:, None] == np.arange(NB)[None, :]).astype(np.float32)
    g2 = (np.arange(NB)[:, None] == p_b[None, :]).astype(np.float32)
    return dict(oddm=oddm, evpmfz=evpmfz, fzp=fzp, liveb=liveb, biasb=biasb,
                initm=initm, g1=g1, g2=g2)


def kernel(attn_logprob, in_lens, out_lens):
    x = np.ascontiguousarray(np.asarray(attn_logprob, np.float32)[:, 0])  # [128,900,200]
    il = np.asarray(in_lens).astype(np.int64)
    ol = np.asarray(out_lens).astype(np.int64)
    Bfull = x.shape[0]
    Tmax = int(ol.max())
    G = (Tmax + 7) // 8
    measures = [t for t in range(4, Tmax - 1 - LAG, RESC)]
    NR = len(measures)

    nc = _build(Tmax, measures, G, NR)

    in_maps = []
    for c in range(NCORES):
        sl = slice(c * NB, (c + 1) * NB)
        m = {"x": np.ascontiguousarray(x[sl]).astype(ml_dtypes.bfloat16)}
        m.update(_host_constants(il[sl], ol[sl], Tmax, G))
        in_maps.append(m)

    import os
    global LAST_RESULTS, LAST_EXEC_S
    LAST_EXEC_S = None
    LAST_RESULTS = run_bass_kernel_spmd(nc, in_maps, list(range(NCORES)))
    res = LAST_RESULTS.results
    if os.environ.get("BASS_PROFILE", "0") == "1":
        try:
            tdir = os.environ.get("BASS_TRACE_DIR") or None
            tr = run_bass_kernel_spmd(nc, in_maps, list(range(NCORES)),
                                      trace=True, tmpdir=tdir)
            if tr.exec_time_ns is not None:
                LAST_EXEC_S = tr.exec_time_ns * 1e-9
                LAST_RESULTS = tr
        except Exception as e:
            print("trace run failed:", e)
        if LAST_EXEC_S is None:
            import time
            ts = []
            for _ in range(3):
                t0 = time.time()
                run_bass_kernel_spmd(nc, in_maps, list(range(NCORES)))
                ts.append(time.time() - t0)
            LAST_EXEC_S = min(ts)

    # host reconstruction
    losses = []
    for c in range(NCORES):
        sl = slice(c * NB, (c + 1) * NB)
        alpha = np.asarray(res[c]["alpha"], np.float64)    # [128, TW]
        outsb = np.asarray(res[c]["outsb"], np.float64)    # [16, NR]
        ss = np.asarray(res[c]["ssden"], np.float64)       # [128, G]
        Lc = il[sl]
        olc = ol[sl]
        lb = np.zeros(NB)
        for b in range(NB):
            # denominators for frames t < ol_b
            ts_ = np.arange(olc[b])
            D = ss[(ts_ % 8) * NB + b, ts_ // 8] + EB
            lnD = np.sum(np.log(np.maximum(D, 1e-300)))
            lnM = np.sum(np.log(np.maximum(outsb[b, :NR], 1e-300)))
            s_hi = 2 * Lc[b]
            v = alpha[b * NCH + s_hi // W, GW + s_hi % W]
            if olc[b] == Tmax:
                s_lo = s_hi - 1
                v = v + alpha[b * NCH + s_lo // W, GW + s_lo % W]
            with np.errstate(divide="ignore", invalid="ignore"):
                ln_true = np.log(v) - lnD + lnM
            loss = -ln_true / Lc[b]
            if not np.isfinite(loss) or loss > 1e20:
                loss = 0.0
            lb[b] = loss
        losses.append(lb)
    return np.float32(np.mean(np.concatenate(losses)[:Bfull]))
